# revision 1
# baseline (speedup 1.0000x reference)
"""Trainium2 Bass kernel for nn_AttnFPN (conv pyramid + 4-layer transformer
decoder with banded self-attention + dense cross-attention over a conv memory).

Sharding: 8 cores = 2 batches x 4 window-quarters of the concatenated pyramid
row space (1920 rows). Each core computes the full conv pyramid for its batch,
selects a 512-row window (480 owned rows + 16-row halo each side) via a
one-hot selection matmul, runs all 4 decoder layers on the window (halo
shrink absorbs the banded self-attention's +-4 reach per layer), and emits its
480 owned rows. The host assembles the [B, 256, 1920] output.

On-chip layout is feature-major throughout: activations live as X^T
[d on partitions (2x128 chunks), rows on free dim]. Softmax runs without max
subtraction (scores empirically bounded; exp(-1e9) underflows to 0 for the
band mask). Softmax denominators are produced as 32-row broadcasts by an
all-ones stationary matmul sharing the AV PSUM bank via column tiling.
"""
import os
import sys

for _p in ('/opt/trn_rl_repo', os.path.expanduser('~/.axon_site/_ro/trn_rl_repo')):
    if os.path.isdir(_p) and _p not in sys.path:
        sys.path.insert(0, _p)

import ml_dtypes
import numpy as np

import concourse.bass as bass
import concourse.mybir as mybir
import concourse.tile as tile
from concourse import bacc
from concourse.bass_utils import run_bass_kernel_spmd
from concourse.masks import make_identity

F32 = mybir.dt.float32
BF16 = mybir.dt.bfloat16
AF = mybir.ActivationFunctionType
OP = mybir.AluOpType

# problem constants
B, CIN, COUT, T, NLV, NLY, H, DFF, KBAND = 2, 512, 256, 2048, 4, 4, 8, 1024, 9
HD = COUT // H           # 32
RW = 512                 # per-core window rows
OWN = 480
HALO = 16
LVL_SIZES = [1024, 512, 256, 128]
LVL_STARTS = [0, 1024, 1536, 1792]
TOT = 1920
NBLK = TOT // 128        # 15 row-blocks of the concat pyramid
XP = 2056                # padded x length (col j holds x[:, j-1], col 0 = zero)
# self-attn subtiles: (q_start, q_len, k_start) window-local
SUBTILES = [(0, 120, 0), (120, 120, 116), (240, 120, 236), (360, 120, 356), (480, 32, 384)]

# ---------------------------------------------------------------------------
# device program
# ---------------------------------------------------------------------------


def _build_nc():
    nc = bacc.Bacc("TRN2", target_bir_lowering=False, debug=False, num_devices=8)

    def din(name, shape):
        return nc.dram_tensor(name, list(shape), F32, kind="ExternalInput")

    t_x = din("xp", [4, 128, XP])                 # x padded, feature chunks
    t_ssel = din("ssel", [NBLK, 128, RW])         # one-hot selection
    t_pe = nc.dram_tensor("pe", [128, 2, RW], BF16, kind="ExternalInput")                # sqrt(C)*0 + PE slice, chunked
    t_smask = nc.dram_tensor("smask", [5, 128, 256], BF16, kind="ExternalInput")        # additive self masks per subtile
    # conv weights (lhsT layouts [i-chunk 128, o])
    t_w1x1 = din("w1x1", [4, 128, 256])
    t_wn0 = din("wn0", [3, 4, 128, 256])          # tap, cc, 128, 256
    t_wnk = din("wnk", [3, 3, 2, 128, 256])       # lvl-1, tap, cc, 128, 256
    t_cb = din("cbias", [128, 2])                 # conv_b (mem)
    t_nb = din("nbias", [4, 128, 2])              # neck biases lvl 0..3
    # per-layer transformer weights
    t_sa_w = din("sa_w", [NLY, 2, 128, 768])      # qkv (q pre-scaled)
    t_sa_b = din("sa_b", [NLY, 128, 6])           # q0 q1 k0 k1 v0 v1
    t_sa_ow = din("sa_ow", [NLY, 2, 128, 256])
    t_sa_ob = din("sa_ob", [NLY, 128, 2])
    t_ca_qw = din("ca_qw", [NLY, 2, 128, 256])    # pre-scaled
    t_ca_qb = din("ca_qb", [NLY, 128, 2])
    t_ca_kw = din("ca_kw", [NLY, 2, 128, 256])
    t_ca_kb = din("ca_kb", [NLY, 128, 2])
    t_ca_vw = din("ca_vw", [NLY, 2, 128, 512])
    t_ca_vbf = din("ca_vbf", [NLY, 1, 512])       # v bias + ones as free-row
    t_ca_ow = din("ca_ow", [NLY, 2, 128, 256])
    t_ca_ob = din("ca_ob", [NLY, 128, 2])
    t_ff1w = din("ff1w", [NLY, 2, 128, 1024])
    t_ff1b = din("ff1b", [NLY, 128, 8])
    t_ff2w = din("ff2w", [NLY, 8, 128, 256])
    t_ff2b = din("ff2b", [NLY, 128, 2])
    t_lngf = din("lngf", [NLY, 1, 3, 256])        # gamma free-rows
    t_lnbf = din("lnbf", [NLY, 1, 3, 256])        # beta free-rows
    t_lngp = din("lngp", [NLY, 128, 6, 2])        # gamma/beta partition-chunks
    t_out = nc.dram_tensor("out", [128, 2, RW], F32, kind="ExternalOutput")

    with tile.TileContext(nc) as tc:
        _emit(nc, tc, locals())
    nc.compile()
    return nc


def _emit(nc, tc, t):
    from contextlib import ExitStack
    ctx = ExitStack()
    with ctx:
        P = 128
        persist = ctx.enter_context(tc.tile_pool(name="persist", bufs=1))
        state = ctx.enter_context(tc.tile_pool(name="state", bufs=3))
        big = ctx.enter_context(tc.tile_pool(name="big", bufs=1))
        kvp = ctx.enter_context(tc.tile_pool(name="kvp", bufs=2))
        wroll = ctx.enter_context(tc.tile_pool(name="wroll", bufs=4))
        wb = ctx.enter_context(tc.tile_pool(name="wb", bufs=1))
        work = ctx.enter_context(tc.tile_pool(name="work", bufs=2))
        stats = ctx.enter_context(tc.tile_pool(name="stats", bufs=1))
        act = ctx.enter_context(tc.tile_pool(name="act", bufs=1))
        epool = ctx.enter_context(tc.tile_pool(name="epool", bufs=5))
        psc = ctx.enter_context(tc.tile_pool(name="psc", bufs=2, space="PSUM"))
        pav = ctx.enter_context(tc.tile_pool(name="pav", bufs=2, space="PSUM"))
        pms = ctx.enter_context(tc.tile_pool(name="pms", bufs=2, space="PSUM"))

        def wslice(dram_ap, tag="w"):
            """Stream a weight slice [*, 128, w<=256] (c-chunked) into SBUF."""
            shp = dram_ap.shape
            if len(shp) == 3:
                tile_ = wroll.tile([P, shp[0], shp[2]], F32, tag=tag)
                nc.sync.dma_start(out=tile_[:], in_=dram_ap.rearrange("c p f -> p c f"))
            else:
                tile_ = wroll.tile([P, shp[1]], F32, tag=tag)
                nc.sync.dma_start(out=tile_[:], in_=dram_ap)
            return tile_

        def stride2(ap3, cc, s, w):
            return ap3[:, cc, s:s + 2 * w].rearrange("p (n two) -> p two n", two=2)[:, 0, :]

        # ---- constants ----
        ident = persist.tile([P, P], F32)
        make_identity(nc, ident[:])
        ident_b = persist.tile([P, P], BF16)
        nc.vector.tensor_copy(ident_b[:], ident[:])
        invn = persist.tile([P, P], F32)
        nc.gpsimd.memset(invn[:], 1.0 / COUT)
        ones_f = persist.tile([1, P], F32)
        nc.gpsimd.memset(ones_f[:], 1.0)
        eps_t = persist.tile([1, 1], F32)
        nc.gpsimd.memset(eps_t[:], 1e-5)

        pe_sb = persist.tile([P, 2, RW], BF16)
        nc.sync.dma_start(out=pe_sb[:], in_=t['t_pe'].ap())

        memT = persist.tile([P, 2, T], F32)

        # ================= pyramid =================
        with tc.tile_pool(name="pyr", bufs=1) as pyr, \
             tc.tile_pool(name="selw", bufs=2) as selw:
            xT = pyr.tile([P, 4, XP], F32)
            nc.sync.dma_start(out=xT[:], in_=t['t_x'].ap().rearrange("c p f -> p c f"))
            cb = pyr.tile([P, 2], F32)
            nc.sync.dma_start(out=cb[:], in_=t['t_cb'].ap())
            nb = pyr.tile([P, 4, 2], F32)
            nc.sync.dma_start(out=nb[:], in_=t['t_nb'].ap().rearrange("l p f -> p l f"))

            # mem = relu(1x1 conv), fc processed in pairs (2 psum slots)
            for oc in range(2):
                for fp in range(2):
                    pss = [pms.tile([P, 512], F32, tag="m", name=f"mempp{oc}_{fp}_{i2}") for i2 in range(2)]
                    for cc in range(4):
                        wsl = wslice(t['t_w1x1'][cc, :, 128 * oc:128 * (oc + 1)])
                        for i, fc in enumerate((2 * fp, 2 * fp + 1)):
                            nc.tensor.matmul(pss[i][:], wsl[:],
                                             xT[:, cc, 1 + 512 * fc:1 + 512 * (fc + 1)],
                                             start=(cc == 0), stop=(cc == 3))
                    for i, fc in enumerate((2 * fp, 2 * fp + 1)):
                        nc.vector.tensor_scalar(out=memT[:, oc, 512 * fc:512 * (fc + 1)],
                                                in0=pss[i][:], scalar1=cb[:, oc:oc + 1],
                                                scalar2=0.0, op0=OP.add, op1=OP.max)

            # neck pyramid (feature-major, 1-col zero pad left)
            lvl_len = [1024, 512, 256, 128]
            lbufs = []
            src_buf = None
            for lv in range(4):
                L = lvl_len[lv]
                lb = pyr.tile([P, 2, L + 8], F32, tag=f"lb{lv}", name=f"lb{lv}")
                nc.gpsimd.memset(lb[:], 0.0)
                lbufs.append(lb)
                n_cc = 4 if lv == 0 else 2
                nfc = (L + 511) // 512
                for oc in range(2):
                    pss = [pms.tile([P, 512], F32, tag="m", name=f"cvp{lv}_{oc}_{i2}") for i2 in range(nfc)]
                    k = 0
                    for cc in range(n_cc):
                        for tap in range(3):
                            if lv == 0:
                                wsl = wslice(t['t_wn0'][tap, cc, :, 128 * oc:128 * (oc + 1)])
                            else:
                                wsl = wslice(t['t_wnk'][lv - 1, tap, cc, :, 128 * oc:128 * (oc + 1)])
                            for fc in range(nfc):
                                w = min(512, L - 512 * fc)
                                rhs = (stride2(xT, cc, 1024 * fc + tap, w) if lv == 0
                                       else stride2(src_buf, cc, 1024 * fc + tap, w))
                                nc.tensor.matmul(pss[fc][:, :w], wsl[:], rhs,
                                                 start=(k == 0), stop=(k == 3 * n_cc - 1))
                            k += 1
                    for fc in range(nfc):
                        w = min(512, L - 512 * fc)
                        nc.vector.tensor_scalar(out=lb[:, oc, 1 + 512 * fc:1 + 512 * fc + w],
                                                in0=pss[fc][:, :w], scalar1=nb[:, lv, oc:oc + 1],
                                                scalar2=0.0, op0=OP.add, op1=OP.max)
                src_buf = lb

            # transpose + selection, per feature chunk
            f0 = state.tile([P, 2, RW], F32, tag="fT")
            blk_of = []
            for lv in range(4):
                for j in range(lvl_len[lv] // 128):
                    blk_of.append((lv, j))
            for dc in range(2):
                sel_ps = pms.tile([P, 512], F32, tag="m")
                for b, (lv, j) in enumerate(blk_of):
                    tr_ps = pms.tile([P, P], F32, tag="m")
                    nc.tensor.transpose(tr_ps[:],
                                        lbufs[lv][:, dc, 1 + 128 * j:1 + 128 * (j + 1)],
                                        ident[:])
                    fr = selw.tile([P, P], F32, tag="frow")
                    nc.vector.tensor_copy(fr[:], tr_ps[:])
                    sb = selw.tile([P, RW], F32, tag="ssel")
                    nc.sync.dma_start(out=sb[:], in_=t['t_ssel'][b])
                    nc.tensor.matmul(sel_ps[:], fr[:], sb[:],
                                     start=(b == 0), stop=(b == NBLK - 1))
                nc.vector.scalar_tensor_tensor(out=f0[:, dc, :], in0=sel_ps[:],
                                               scalar=float(np.sqrt(COUT)),
                                               in1=pe_sb[:, dc, :],
                                               op0=OP.mult, op1=OP.add)

        # ================= decoder layers =================
        import os as _os
        _nlayers = int(_os.environ.get('KERN_NLAYERS', str(NLY)))
        _stage = _os.environ.get('KERN_STAGE', 'all')
        fT = f0
        for l in range(_nlayers):
            b_sa = wb.tile([P, 6], F32, tag="b_sa")
            nc.sync.dma_start(out=b_sa[:], in_=t['t_sa_b'][l])
            b_sao = wb.tile([P, 2], F32, tag="b_sao")
            nc.sync.dma_start(out=b_sao[:], in_=t['t_sa_ob'][l])
            b_caq = wb.tile([P, 2], F32, tag="b_caq")
            nc.sync.dma_start(out=b_caq[:], in_=t['t_ca_qb'][l])
            b_cak = wb.tile([P, 2], F32, tag="b_cak")
            nc.sync.dma_start(out=b_cak[:], in_=t['t_ca_kb'][l])
            vb_f = wb.tile([1, 512], F32, tag="vb_f")
            nc.sync.dma_start(out=vb_f[:], in_=t['t_ca_vbf'][l])
            b_cao = wb.tile([P, 2], F32, tag="b_cao")
            nc.sync.dma_start(out=b_cao[:], in_=t['t_ca_ob'][l])
            b_ff1 = wb.tile([P, 8], F32, tag="b_ff1")
            nc.sync.dma_start(out=b_ff1[:], in_=t['t_ff1b'][l])
            b_ff2 = wb.tile([P, 2], F32, tag="b_ff2")
            nc.sync.dma_start(out=b_ff2[:], in_=t['t_ff2b'][l])
            gf = bf = None
            gp = wb.tile([P, 6, 2], F32, tag="gp")
            nc.sync.dma_start(out=gp[:], in_=t['t_lngp'][l])

            # ---- cross K / V (independent of f) ----
            KT = kvp.tile([P, 2, T], BF16, tag="KT")
            for oc in range(2):
                wk = wslice(t['t_ca_kw'][l, :, :, 128 * oc:128 * (oc + 1)])
                for tp2 in range(2):
                    ps = psc.tile([P, 1024], F32, tag="sc")
                    for half in range(2):
                        tck = 2 * tp2 + half
                        for ic in range(2):
                            nc.tensor.matmul(ps[:, 512 * half:512 * (half + 1)],
                                             wk[:, ic, :],
                                             memT[:, ic, 512 * tck:512 * (tck + 1)],
                                             start=(ic == 0), stop=(ic == 1))
                    nc.vector.tensor_scalar_add(KT[:, oc, 1024 * tp2:1024 * (tp2 + 1)],
                                                ps[:], b_cak[:, oc:oc + 1])
            Vp = kvp.tile([P, 16, 512], BF16, tag="Vp")
            wv = wb.tile([P, 2, 512], F32, tag="wv")
            nc.sync.dma_start(out=wv[:], in_=t['t_ca_vw'][l].rearrange("c p f -> p c f"))
            for kc in range(16):
                ps = pms.tile([P, 512], F32, tag="m")
                for ic in range(2):
                    nc.tensor.matmul(ps[:], memT[:, ic, 128 * kc:128 * (kc + 1)],
                                     wv[:, ic, :], start=(ic == 0), stop=False)
                nc.tensor.matmul(ps[:], ones_f[:], vb_f[:], start=False, stop=True)
                nc.vector.tensor_copy(Vp[:, kc, :], ps[:])

            if _stage == 'kv':
                continue
            # ---- self attention ----
            QTs = act.tile([P, 2, RW], F32, tag="QTs")
            KTs = act.tile([P, 2, RW], F32, tag="KTs")
            VTs = act.tile([P, 2, RW], F32, tag="VTs")
            for wi, dst in ((0, QTs), (1, KTs), (2, VTs)):
                for oc in range(2):
                    wsl = wslice(t['t_sa_w'][l, :, :, 256 * wi + 128 * oc:256 * wi + 128 * (oc + 1)])
                    ps = pms.tile([P, 512], F32, tag="m")
                    for ic in range(2):
                        nc.tensor.matmul(ps[:], wsl[:, ic, :], fT[:, ic, :],
                                         start=(ic == 0), stop=(ic == 1))
                    nc.vector.tensor_scalar_add(dst[:, oc, :], ps[:],
                                                b_sa[:, 2 * wi + oc:2 * wi + oc + 1])
            OsT = act.tile([P, 2, RW], F32, tag="OT")
            for sti, (qs, ql, ks) in enumerate(SUBTILES):
                vst_ps = pms.tile([P, 256], F32, tag="m")
                for hc in range(2):
                    nc.tensor.transpose(vst_ps[:, 128 * hc:128 * (hc + 1)],
                                        VTs[:, hc, ks:ks + 128], ident[:])
                vst = work.tile([P, 512], BF16, tag="vst")
                vst_v = vst[:].rearrange("p (h two j) -> p h two j", h=8, two=2)
                nc.gpsimd.memset(vst_v[:, :, 1, :], 1.0)
                nc.vector.tensor_copy(vst_v[:, :, 0, :], vst_ps[:])
                mskt = work.tile([P, 256], BF16, tag="mskt")
                nc.sync.dma_start(out=mskt[:], in_=t['t_smask'][sti])
                for p in range(4):
                    h0, h1 = 2 * p, 2 * p + 1
                    sps = psc.tile([P, 1024], F32, tag="sc")
                    for hi, hh in enumerate((h0, h1)):
                        nc.tensor.matmul(sps[:, 512 * hi:512 * hi + ql],
                                         KTs[32 * (hh % 4):32 * (hh % 4) + 32, hh // 4, ks:ks + 128],
                                         QTs[32 * (hh % 4):32 * (hh % 4) + 32, hh // 4, qs:qs + ql],
                                         start=True, stop=False,
                                         tile_position=(32 * (hh % 4), 0))
                    for hi in range(2):
                        nc.tensor.matmul(sps[:, 512 * hi:512 * hi + ql], ident_b[:],
                                         mskt[:, 128 * hi:128 * hi + ql],
                                         start=False, stop=True)
                    spv = sps[:].rearrange("p (b q) -> p b q", b=2)[:, :, 0:ql]
                    es = epool.tile([P, 256], BF16, tag="E")
                    esv = es[:].rearrange("p (b q) -> p b q", b=2)[:, :, 0:ql]
                    nc.scalar.activation(out=esv, in_=spv, func=AF.Exp)
                    avp = pav.tile([P, 512], F32, tag="av")
                    nc.tensor.matmul(avp[0:64, :ql], vst[:, 64 * h0:64 * h0 + 64],
                                     es[:, 0:ql], start=True, stop=True,
                                     tile_position=(0, 0))
                    nc.tensor.matmul(avp[64:128, :ql], vst[:, 64 * h1:64 * h1 + 64],
                                     es[:, 128:128 + ql], start=True, stop=True,
                                     tile_position=(0, 64))
                    zr = work.tile([P, 512], F32, tag="zr")
                    nc.vector.reciprocal(zr[:, :ql], avp[:, :ql])
                    nc.vector.tensor_mul(OsT[64 * (p % 2):64 * (p % 2) + 32, p // 2, qs:qs + ql],
                                         avp[0:32, :ql], zr[32:64, :ql])
                    nc.vector.tensor_mul(OsT[64 * (p % 2) + 32:64 * (p % 2) + 64, p // 2, qs:qs + ql],
                                         avp[64:96, :ql], zr[96:128, :ql])
            R1 = state.tile([P, 2, RW], F32, tag="fT")
            for oc in range(2):
                wsl = wslice(t['t_sa_ow'][l, :, :, 128 * oc:128 * (oc + 1)])
                ps = pms.tile([P, 512], F32, tag="m")
                for ic in range(2):
                    nc.tensor.matmul(ps[:], wsl[:, ic, :], OsT[:, ic, :],
                                     start=(ic == 0), stop=(ic == 1))
                nc.vector.scalar_tensor_tensor(out=R1[:, oc, :], in0=ps[:],
                                               scalar=b_sao[:, oc:oc + 1], in1=fT[:, oc, :],
                                               op0=OP.add, op1=OP.add)
            f1 = state.tile([P, 2, RW], F32, tag="fT")
            _layernorm(nc, tc, pms, work, stats, act, R1, f1, gf, bf, gp, 0, invn, ones_f, None, eps_t)

            if _stage == 'self':
                fT = f1
                continue
            # ---- cross attention ----
            QTc = act.tile([P, 2, RW], BF16, tag="QTc")
            for oc in range(2):
                wsl = wslice(t['t_ca_qw'][l, :, :, 128 * oc:128 * (oc + 1)])
                ps = pms.tile([P, 512], F32, tag="m")
                for ic in range(2):
                    nc.tensor.matmul(ps[:], wsl[:, ic, :], f1[:, ic, :],
                                     start=(ic == 0), stop=(ic == 1))
                nc.vector.tensor_scalar_add(QTc[:, oc, :], ps[:], b_caq[:, oc:oc + 1])
            OcT = act.tile([P, 2, RW], F32, tag="OT")
            for p in range(4):
                h0, h1 = 2 * p, 2 * p + 1
                avp = pav.tile([P, 512], F32, tag="av")
                for kc in range(16):
                    scp = psc.tile([P, 1024], F32, tag="sc")
                    for hi, hh in enumerate((h0, h1)):
                        nc.tensor.matmul(scp[:, 512 * hi:512 * (hi + 1)],
                                         KT[32 * (hh % 4):32 * (hh % 4) + 32, hh // 4, 128 * kc:128 * (kc + 1)],
                                         QTc[32 * (hh % 4):32 * (hh % 4) + 32, hh // 4, :],
                                         start=True, stop=True, tile_position=(32 * (hh % 4), 0))
                    ec = epool.tile([P, 1024], BF16, tag="E")
                    nc.scalar.activation(out=ec[:], in_=scp[:], func=AF.Exp)
                    st, sp = (kc == 0), (kc == 15)
                    nc.tensor.matmul(avp[0:64, :], Vp[:, kc, 64 * h0:64 * h0 + 64],
                                     ec[:, 0:512], start=st, stop=sp, tile_position=(0, 0))
                    nc.tensor.matmul(avp[64:128, :], Vp[:, kc, 64 * h1:64 * h1 + 64],
                                     ec[:, 512:1024], start=st, stop=sp, tile_position=(0, 64))
                zr = work.tile([P, 512], F32, tag="zr")
                nc.vector.reciprocal(zr[:, :], avp[:, :])
                nc.vector.tensor_mul(OcT[64 * (p % 2):64 * (p % 2) + 32, p // 2, :],
                                     avp[0:32, :], zr[32:64, :])
                nc.vector.tensor_mul(OcT[64 * (p % 2) + 32:64 * (p % 2) + 64, p // 2, :],
                                     avp[64:96, :], zr[96:128, :])
            R2 = state.tile([P, 2, RW], F32, tag="fT")
            for oc in range(2):
                wsl = wslice(t['t_ca_ow'][l, :, :, 128 * oc:128 * (oc + 1)])
                ps = pms.tile([P, 512], F32, tag="m")
                for ic in range(2):
                    nc.tensor.matmul(ps[:], wsl[:, ic, :], OcT[:, ic, :],
                                     start=(ic == 0), stop=(ic == 1))
                nc.vector.scalar_tensor_tensor(out=R2[:, oc, :], in0=ps[:],
                                               scalar=b_cao[:, oc:oc + 1], in1=f1[:, oc, :],
                                               op0=OP.add, op1=OP.add)
            f2 = state.tile([P, 2, RW], F32, tag="fT")
            _layernorm(nc, tc, pms, work, stats, act, R2, f2, gf, bf, gp, 1, invn, ones_f, None, eps_t)

            if _stage == 'cross':
                fT = f2
                continue
            # ---- ffn (Hb in 2 halves of 4) ----
            ps_oc = [pms.tile([P, 512], F32, tag="m", name=f"ffp{l}_{i2}") for i2 in range(2)]
            for hf in range(2):
                Hb = big.tile([P, 4, RW], F32, tag="Hb")
                for jj in range(4):
                    j = 4 * hf + jj
                    w1 = wslice(t['t_ff1w'][l, :, :, 128 * j:128 * (j + 1)])
                    ps = psc.tile([P, 1024], F32, tag="sc")
                    for ic in range(2):
                        nc.tensor.matmul(ps[:, 0:512], w1[:, ic, :], f2[:, ic, :],
                                         start=(ic == 0), stop=(ic == 1))
                    nc.vector.tensor_scalar(out=Hb[:, jj, :], in0=ps[:, 0:512],
                                            scalar1=b_ff1[:, j:j + 1], scalar2=0.0,
                                            op0=OP.add, op1=OP.max)
                for jj in range(4):
                    j = 4 * hf + jj
                    w2 = wslice(t['t_ff2w'][l, j], tag="w256")
                    for oc in range(2):
                        nc.tensor.matmul(ps_oc[oc][:], w2[:, 128 * oc:128 * (oc + 1)],
                                         Hb[:, jj, :], start=(j == 0), stop=(j == 7))
            R3 = state.tile([P, 2, RW], F32, tag="fT")
            for oc in range(2):
                nc.vector.scalar_tensor_tensor(out=R3[:, oc, :], in0=ps_oc[oc][:],
                                               scalar=b_ff2[:, oc:oc + 1], in1=f2[:, oc, :],
                                               op0=OP.add, op1=OP.add)
            f3 = state.tile([P, 2, RW], F32, tag="fT")
            _layernorm(nc, tc, pms, work, stats, act, R3, f3, gf, bf, gp, 2, invn, ones_f, None, eps_t)
            fT = f3

        nc.sync.dma_start(out=t['t_out'].ap(), in_=fT[:])


def _layernorm(nc, tc, pms, work, stats, act, R, out, gf, bf, gp, which, invn, ones_f, negones_row, eps_t):
    """Feature-major LN over d=256 (2 partition chunks), rows on free dim.
    Stats via all-(1/256) stationary matmuls producing 128-row broadcasts.
    gamma/beta are per-partition scalars (gp packs both: col 2*which=gamma,
    handled by caller passing slices)."""
    P = 128
    sq = act.tile([P, 2, RW], F32, tag="sq")
    nc.vector.tensor_mul(sq[:, 0, :], R[:, 0, :], R[:, 0, :])
    nc.vector.tensor_mul(sq[:, 1, :], R[:, 1, :], R[:, 1, :])
    mB = pms.tile([P, 512], F32, tag="m")
    for ic in range(2):
        nc.tensor.matmul(mB[:], invn[:], R[:, ic, :], start=(ic == 0), stop=(ic == 1))
    msB = pms.tile([P, 512], F32, tag="m")
    for ic in range(2):
        nc.tensor.matmul(msB[:], invn[:], sq[:, ic, :], start=(ic == 0), stop=(ic == 1))
    m2 = stats.tile([1, RW], F32, tag="s1")
    nc.scalar.activation(out=m2[:], in_=mB[0:1, :], func=AF.Square)
    var = stats.tile([1, RW], F32, tag="s2")
    nc.vector.scalar_tensor_tensor(out=var[:], in0=msB[0:1, :], scalar=1.0,
                                   in1=m2[:], op0=OP.mult, op1=OP.subtract)
    sd = stats.tile([1, RW], F32, tag="s1")
    nc.scalar.activation(out=sd[:], in_=var[:], func=AF.Sqrt, bias=eps_t[:])
    r = stats.tile([1, RW], F32, tag="s2")
    nc.vector.reciprocal(r[:], sd[:])
    rB = pms.tile([P, 512], F32, tag="m")
    nc.tensor.matmul(rB[:], ones_f[:], r[:], start=True, stop=True)
    for oc in range(2):
        # c = (R - mB) * rB ; out = c * gamma + beta   (gamma/beta per-partition)
        c = work.tile([P, RW], F32, tag="tmp")
        nc.vector.tensor_sub(c[:], R[:, oc, :], mB[:])
        d = work.tile([P, RW], F32, tag="tmp2")
        nc.vector.tensor_mul(d[:], c[:], rB[:])
        nc.vector.tensor_scalar(out=out[:, oc, :], in0=d[:],
                                scalar1=gp[:, 2 * which, oc:oc + 1],
                                scalar2=gp[:, 2 * which + 1, oc:oc + 1],
                                op0=OP.mult, op1=OP.add)


# ---------------------------------------------------------------------------
# host side
# ---------------------------------------------------------------------------

def _sinusoidal_pe(t, d):
    pos = np.arange(t, dtype=np.float32)[:, None]
    div = np.exp(np.arange(0, d, 2, dtype=np.float32) * (-np.log(10000.0) / d))
    ang = pos * div
    pe = np.zeros((t, d), np.float32)
    pe[:, 0::2] = np.sin(ang)
    pe[:, 1::2] = np.cos(ang)
    return pe


def _concat_row_to_level(r):
    for li in range(NLV):
        if r < LVL_STARTS[li] + LVL_SIZES[li]:
            return li, r - LVL_STARTS[li]
    raise ValueError(r)


def _core_meta(c):
    w0 = OWN * c - HALO
    S = np.zeros((TOT, RW), np.float32)
    valid = np.zeros(RW, bool)
    lvl_of = np.full(RW, -1)
    pos_of = np.full(RW, -1)
    for j in range(RW):
        r = w0 + j
        if 0 <= r < TOT:
            S[r, j] = 1.0
            valid[j] = True
            lvl_of[j], pos_of[j] = _concat_row_to_level(r)
    pes = [_sinusoidal_pe(sz, COUT) for sz in LVL_SIZES]
    pe_plus = np.zeros((COUT, RW), np.float32)
    for j in range(RW):
        if valid[j]:
            pe_plus[:, j] = pes[lvl_of[j]][pos_of[j]]
    smask = np.full((5, 128, 256), -1e9, np.float32)
    for sti, (qs, ql, ks) in enumerate(SUBTILES):
        m = np.full((128, ql), -1e9, np.float32)
        for jq in range(ql):
            q = qs + jq
            for jk in range(128):
                k = ks + jk
                if k >= RW:
                    continue
                if valid[q] and valid[k]:
                    if lvl_of[q] == lvl_of[k] and abs(pos_of[q] - pos_of[k]) <= KBAND // 2:
                        m[jk, jq] = 0.0
                elif (not valid[q]) and k == q:
                    m[jk, jq] = 0.0
        smask[sti, :, 0:ql] = m
        smask[sti, :, 128:128 + ql] = m
    return S, pe_plus, smask


def _chunk_p(v):
    """[n*128] -> [128, n] partition-major."""
    v = np.asarray(v, np.float32)
    n = v.shape[0] // 128
    return v.reshape(n, 128).T.copy()


def _lhsT(w):
    """[O, I] weight -> [n_ic, 128, O] lhsT chunks (W^T chunked over I)."""
    wT = np.ascontiguousarray(np.asarray(w, np.float32).T)  # [I, O]
    I = wT.shape[0]
    return wT.reshape(I // 128, 128, wT.shape[1])


_NC_CACHE = None
LAST_EXEC_NS = None


def _get_nc():
    global _NC_CACHE
    if _NC_CACHE is None:
        _NC_CACHE = _build_nc()
    return _NC_CACHE


def _prepare_in_maps(inputs):
    inp = {k: np.asarray(v, np.float32) for k, v in inputs.items()}

    scale = 1.0 / np.sqrt(HD)
    common = {}
    common['w1x1'] = _lhsT(inp['conv_w'][:, :, 0])
    common['wn0'] = np.stack([_lhsT(inp['neck_w0'][:, :, tp]) for tp in range(3)])
    common['wnk'] = np.stack([np.stack([_lhsT(inp['neck_w'][lv][:, :, tp]) for tp in range(3)])
                              for lv in range(3)])
    common['cbias'] = _chunk_p(inp['conv_b'])
    common['nbias'] = np.stack([_chunk_p(inp['neck_b0'])] +
                               [_chunk_p(inp['neck_b'][i]) for i in range(3)])

    sa_w, sa_b = [], []
    for l in range(NLY):
        w = inp['sa_in_w'][l].copy()    # [768, 256]
        b = inp['sa_in_b'][l].copy()
        w[:COUT] *= scale
        b[:COUT] *= scale
        sa_w.append(_lhsT(w))           # [2, 128, 768]
        bb = np.zeros((128, 6), np.float32)
        for wi in range(3):
            bb[:, 2 * wi:2 * wi + 2] = _chunk_p(b[wi * COUT:(wi + 1) * COUT])
        sa_b.append(bb)
    common['sa_w'] = np.stack(sa_w)
    common['sa_b'] = np.stack(sa_b)
    common['sa_ow'] = np.stack([_lhsT(inp['sa_out_w'][l]) for l in range(NLY)])
    common['sa_ob'] = np.stack([_chunk_p(inp['sa_out_b'][l]) for l in range(NLY)])
    common['ca_qw'] = np.stack([_lhsT(inp['ca_in_w'][l][:COUT] * scale) for l in range(NLY)])
    common['ca_qb'] = np.stack([_chunk_p(inp['ca_in_b'][l][:COUT] * scale) for l in range(NLY)])
    common['ca_kw'] = np.stack([_lhsT(inp['ca_in_w'][l][COUT:2 * COUT]) for l in range(NLY)])
    common['ca_kb'] = np.stack([_chunk_p(inp['ca_in_b'][l][COUT:2 * COUT]) for l in range(NLY)])
    ca_vw, ca_vbf = [], []
    for l in range(NLY):
        wT = _lhsT(inp['ca_in_w'][l][2 * COUT:])          # [2, 128, 256]
        waug = np.zeros((2, 128, 512), np.float32)
        baug = np.zeros((1, 512), np.float32)
        for hh2 in range(H):
            waug[:, :, 64 * hh2:64 * hh2 + 32] = wT[:, :, 32 * hh2:32 * hh2 + 32]
            baug[0, 64 * hh2:64 * hh2 + 32] = inp['ca_in_b'][l][2 * COUT + 32 * hh2:2 * COUT + 32 * (hh2 + 1)]
            baug[0, 64 * hh2 + 32:64 * (hh2 + 1)] = 1.0
        ca_vw.append(waug)
        ca_vbf.append(baug)
    common['ca_vw'] = np.stack(ca_vw)
    common['ca_vbf'] = np.stack(ca_vbf)
    common['ca_ow'] = np.stack([_lhsT(inp['ca_out_w'][l]) for l in range(NLY)])
    common['ca_ob'] = np.stack([_chunk_p(inp['ca_out_b'][l]) for l in range(NLY)])
    common['ff1w'] = np.stack([_lhsT(inp['ff1_w'][l]) for l in range(NLY)])
    common['ff1b'] = np.stack([_chunk_p(inp['ff1_b'][l]) for l in range(NLY)])
    common['ff2w'] = np.stack([_lhsT(inp['ff2_w'][l]) for l in range(NLY)])
    common['ff2b'] = np.stack([_chunk_p(inp['ff2_b'][l]) for l in range(NLY)])
    common['lngf'] = np.stack([np.stack([inp['ln1_g'][l], inp['ln2_g'][l], inp['ln3_g'][l]])
                               for l in range(NLY)]).reshape(NLY, 1, 3, COUT)
    common['lnbf'] = np.stack([np.stack([inp['ln1_b'][l], inp['ln2_b'][l], inp['ln3_b'][l]])
                               for l in range(NLY)]).reshape(NLY, 1, 3, COUT)
    lngp = np.zeros((NLY, 128, 6, 2), np.float32)
    for l in range(NLY):
        for wi, (g, b) in enumerate(((inp['ln1_g'][l], inp['ln1_b'][l]),
                                     (inp['ln2_g'][l], inp['ln2_b'][l]),
                                     (inp['ln3_g'][l], inp['ln3_b'][l]))):
            lngp[l, :, 2 * wi, :] = _chunk_p(g)
            lngp[l, :, 2 * wi + 1, :] = _chunk_p(b)
    common['lngp'] = lngp

    metas = [_core_meta(c) for c in range(4)]
    in_maps = []
    for core in range(8):
        b, c = core // 4, core % 4
        S, pe_plus, smask = metas[c]
        xp = np.zeros((CIN, XP), np.float32)
        xp[:, 1:1 + T] = inp['x'][b]
        m = dict(common)
        m['xp'] = xp.reshape(4, 128, XP)
        m['ssel'] = S.reshape(NBLK, 128, RW)
        m['pe'] = pe_plus.reshape(2, 128, RW).transpose(1, 0, 2).astype(ml_dtypes.bfloat16)
        m['smask'] = smask.astype(ml_dtypes.bfloat16)
        in_maps.append(m)
    return in_maps


def kernel(**inputs):
    nc = _get_nc()
    in_maps = _prepare_in_maps(inputs)

    global LAST_EXEC_NS
    trace = bool(int(os.environ.get('KERN_TRACE', '0')))
    res = run_bass_kernel_spmd(nc, in_maps, list(range(8)), trace=trace)
    if res.exec_time_ns is not None:
        LAST_EXEC_NS = res.exec_time_ns

    out = np.zeros((B, COUT, TOT), np.float32)
    for core in range(8):
        b, c = core // 4, core % 4
        o = res.results[core]['out']          # [128, 2, RW]
        fT = o.transpose(1, 0, 2).reshape(COUT, RW)
        out[b, :, OWN * c:OWN * (c + 1)] = fT[:, HALO:HALO + OWN]
    return out


def bench(n=6, **inputs):
    """Time pure device execution with inputs pre-staged on the 8 cores."""
    import time
    import jax
    from jax.sharding import Mesh, PartitionSpec
    from jax.experimental.shard_map import shard_map
    from concourse import bass2jax, mybir as _mybir

    nc = _get_nc()
    in_maps = _prepare_in_maps(inputs)
    n_cores = 8

    bass2jax.install_neuronx_cc_hook()
    partition_name = nc.partition_id_tensor.name if nc.partition_id_tensor else None
    in_names, out_names, out_avals, zero_outs = [], [], [], []
    for alloc in nc.m.functions[0].allocations:
        if not isinstance(alloc, mybir.MemoryLocationSet):
            continue
        name = alloc.memorylocations[0].name
        if alloc.kind == "ExternalInput":
            if name != partition_name:
                in_names.append(name)
        elif alloc.kind == "ExternalOutput":
            out_names.append(name)
            shape = tuple(alloc.tensor_shape)
            dt = mybir.dt.np(alloc.dtype)
            out_avals.append(jax.core.ShapedArray(shape, dt))
            zero_outs.append(np.zeros(shape, dt))
    n_params, n_outs = len(in_names), len(out_avals)
    all_in_names = in_names + out_names + ([partition_name] if partition_name else [])

    def _body(*args):
        operands = list(args)
        if partition_name is not None:
            operands.append(bass2jax.partition_id_tensor())
        outs = bass2jax._bass_exec_p.bind(
            *operands, out_avals=tuple(out_avals), in_names=tuple(all_in_names),
            out_names=tuple(out_names), lowering_input_output_aliases=(),
            sim_require_finite=True, sim_require_nnan=True, nc=nc)
        return tuple(outs)

    devices = jax.devices()[:n_cores]
    mesh = Mesh(np.asarray(devices), ("core",))
    in_specs = (PartitionSpec("core"),) * (n_params + n_outs)
    out_specs = (PartitionSpec("core"),) * n_outs
    sharded = jax.jit(shard_map(_body, mesh=mesh, in_specs=in_specs,
                                out_specs=out_specs, check_rep=False),
                      keep_unused=True)  # NO donation so buffers are reusable
    from jax.sharding import NamedSharding
    shard = NamedSharding(mesh, PartitionSpec("core"))
    concat_in = [np.concatenate([np.asarray(in_maps[c][nm]) for c in range(n_cores)], axis=0)
                 for nm in in_names]
    concat_zeros = [np.zeros((n_cores * z.shape[0], *z.shape[1:]), z.dtype) for z in zero_outs]
    dev_in = [jax.device_put(a, shard) for a in concat_in]
    dev_zero = [jax.device_put(a, shard) for a in concat_zeros]
    for a in dev_in + dev_zero:
        a.block_until_ready()
    # warmup
    outs = sharded(*dev_in, *dev_zero)
    jax.block_until_ready(outs)
    times = []
    for _ in range(n):
        t0 = time.perf_counter()
        outs = sharded(*dev_in, *dev_zero)
        jax.block_until_ready(outs)
        times.append(time.perf_counter() - t0)
    return times


def timeline_estimate():
    """Cost-model single-core timeline estimate (ns)."""
    from concourse.timeline_sim import TimelineSim
    nc = _get_nc()
    ts = TimelineSim(nc, trace=False)
    ts.simulate()
    return ts



# revision 4
# speedup vs baseline: 1.5205x; 1.5205x over previous
"""Trainium2 Bass kernel for nn_AttnFPN (conv pyramid + 4-layer transformer
decoder with banded self-attention + dense cross-attention over a conv memory).

Sharding: 8 cores = 2 batches x 4 window-quarters of the concatenated pyramid
row space (1920 rows). Each core computes the full conv pyramid for its batch,
selects a 512-row window (480 owned rows + 16-row halo each side) via a
one-hot selection matmul, runs all 4 decoder layers on the window (halo
shrink absorbs the banded self-attention's +-4 reach per layer), and emits its
480 owned rows. The host assembles the [B, 256, 1920] output.

On-chip layout is feature-major throughout: activations live as X^T
[d on partitions (2x128 chunks), rows on free dim]. All matmul operands are
bf16 (PE: 1 cycle/row vs 4 for fp32); PSUM accumulation stays fp32. Softmax
runs without max subtraction (scores empirically bounded; exp(-1e9)
underflows to 0 for the band mask). Softmax denominators are produced as
32-row broadcasts by interleaved all-ones columns in the V stationary tiles
(ones written by memset; cross-attn V bias is folded into the output-proj
bias on the host).
"""
import os
import sys

for _p in ('/opt/trn_rl_repo', os.path.expanduser('~/.axon_site/_ro/trn_rl_repo')):
    if os.path.isdir(_p) and _p not in sys.path:
        sys.path.insert(0, _p)

import ml_dtypes
import numpy as np

import concourse.bass as bass
import concourse.mybir as mybir
import concourse.tile as tile
from concourse import bacc
from concourse.bass_utils import run_bass_kernel_spmd
from concourse.masks import make_identity

F32 = mybir.dt.float32
BF16 = mybir.dt.bfloat16
AF = mybir.ActivationFunctionType
OP = mybir.AluOpType

# problem constants
B, CIN, COUT, T, NLV, NLY, H, DFF, KBAND = 2, 512, 256, 2048, 4, 4, 8, 1024, 9
HD = COUT // H           # 32
RW = 512                 # per-core window rows
OWN = 480
HALO = 16
LVL_SIZES = [1024, 512, 256, 128]
LVL_STARTS = [0, 1024, 1536, 1792]
TOT = 1920
NBLK = TOT // 128        # 15 row-blocks of the concat pyramid
XP = 2056                # padded x length (col j holds x[:, j-1], col 0 = zero)
# self-attn subtiles: (q_start, q_len, k_start) window-local
SUBTILES = [(0, 120, 0), (120, 120, 116), (240, 120, 236), (360, 120, 356), (480, 32, 384)]

# ---------------------------------------------------------------------------
# device program
# ---------------------------------------------------------------------------


def _build_nc():
    nc = bacc.Bacc("TRN2", target_bir_lowering=False, debug=False, num_devices=8)

    def din(name, shape, dt=BF16):
        return nc.dram_tensor(name, list(shape), dt, kind="ExternalInput")

    t_x = din("xp", [4, 128, XP])                 # x padded, feature chunks
    t_ssel = din("ssel", [NBLK, 128, RW])         # one-hot selection
    t_pe = din("pe", [128, 2, RW])                # sqrt(C)*0 + PE slice, chunked
    t_smask = din("smask", [5, 128, 256])         # additive self masks per subtile
    # conv weights (lhsT layouts [i-chunk 128, o])
    t_w1x1 = din("w1x1", [4, 128, 256])
    t_wn0 = din("wn0", [3, 4, 128, 256])          # tap, cc, 128, 256
    t_wnk = din("wnk", [3, 3, 2, 128, 256])       # lvl-1, tap, cc, 128, 256
    t_cb = din("cbias", [128, 2], F32)            # conv_b (mem)
    t_nb = din("nbias", [4, 128, 2], F32)         # neck biases lvl 0..3
    # per-layer transformer weights
    t_sa_w = din("sa_w", [NLY, 2, 128, 768])      # qkv (q pre-scaled)
    t_sa_b = din("sa_b", [NLY, 128, 6], F32)      # q0 q1 k0 k1 v0 v1
    t_sa_ow = din("sa_ow", [NLY, 2, 128, 256])
    t_sa_ob = din("sa_ob", [NLY, 128, 2], F32)
    t_ca_qw = din("ca_qw", [NLY, 2, 128, 256])    # pre-scaled
    t_ca_qb = din("ca_qb", [NLY, 128, 2], F32)
    t_ca_kw = din("ca_kw", [NLY, 2, 128, 256])
    t_ca_kb = din("ca_kb", [NLY, 128, 2], F32)
    t_ca_vw = din("ca_vw", [NLY, 2, 128, 512])    # head-interleaved, ones cols zero
    t_ca_ow = din("ca_ow", [NLY, 2, 128, 256])
    t_ca_ob = din("ca_ob", [NLY, 128, 2], F32)    # includes folded V bias
    t_ff1w = din("ff1w", [NLY, 2, 128, 1024])
    t_ff1b = din("ff1b", [NLY, 128, 8], F32)
    t_ff2w = din("ff2w", [NLY, 8, 128, 256])
    t_ff2b = din("ff2b", [NLY, 128, 2], F32)
    t_lngp = din("lngp", [NLY, 128, 6, 2], F32)   # gamma/beta partition-chunks
    t_out = nc.dram_tensor("out", [128, 2, RW], F32, kind="ExternalOutput")

    with tile.TileContext(nc) as tc:
        _emit(nc, tc, locals())
    nc.compile()
    return nc


def _emit(nc, tc, t):
    from contextlib import ExitStack
    ctx = ExitStack()
    with ctx:
        P = 128
        persist = ctx.enter_context(tc.tile_pool(name="persist", bufs=1))
        state = ctx.enter_context(tc.tile_pool(name="state", bufs=3))
        big = ctx.enter_context(tc.tile_pool(name="big", bufs=1))
        kvp = ctx.enter_context(tc.tile_pool(name="kvp", bufs=2))
        wroll = ctx.enter_context(tc.tile_pool(name="wroll", bufs=4))
        wb = ctx.enter_context(tc.tile_pool(name="wb", bufs=1))
        work = ctx.enter_context(tc.tile_pool(name="work", bufs=2))
        stats = ctx.enter_context(tc.tile_pool(name="stats", bufs=1))
        act = ctx.enter_context(tc.tile_pool(name="act", bufs=1))
        epool = ctx.enter_context(tc.tile_pool(name="epool", bufs=5))
        psc = ctx.enter_context(tc.tile_pool(name="psc", bufs=2, space="PSUM"))
        pav = ctx.enter_context(tc.tile_pool(name="pav", bufs=2, space="PSUM"))
        pms = ctx.enter_context(tc.tile_pool(name="pms", bufs=2, space="PSUM"))

        def wslice(dram_ap, tag="w"):
            """Stream a weight slice [*, 128, w<=256] (c-chunked) into SBUF."""
            shp = dram_ap.shape
            if len(shp) == 3:
                tile_ = wroll.tile([P, shp[0], shp[2]], BF16, tag=tag)
                nc.sync.dma_start(out=tile_[:], in_=dram_ap.rearrange("c p f -> p c f"))
            else:
                tile_ = wroll.tile([P, shp[1]], BF16, tag=tag)
                nc.sync.dma_start(out=tile_[:], in_=dram_ap)
            return tile_

        def stride2(ap3, cc, s, w):
            return ap3[:, cc, s:s + 2 * w].rearrange("p (n two) -> p two n", two=2)[:, 0, :]

        # ---- constants ----
        ident = persist.tile([P, P], F32)
        make_identity(nc, ident[:])
        ident_b = persist.tile([P, P], BF16)
        nc.vector.tensor_copy(ident_b[:], ident[:])
        invn = persist.tile([P, P], BF16)
        nc.gpsimd.memset(invn[:], 1.0 / COUT)
        ones_f = persist.tile([1, P], BF16)
        nc.gpsimd.memset(ones_f[:], 1.0)
        eps_t = persist.tile([1, 1], F32)
        nc.gpsimd.memset(eps_t[:], 1e-5)

        pe_sb = persist.tile([P, 2, RW], BF16)
        nc.sync.dma_start(out=pe_sb[:], in_=t['t_pe'].ap())

        memT = persist.tile([P, 2, T], BF16)

        # ================= pyramid =================
        with tc.tile_pool(name="pyr", bufs=1) as pyr, \
             tc.tile_pool(name="selw", bufs=2) as selw:
            xT = pyr.tile([P, 4, XP], BF16)
            nc.sync.dma_start(out=xT[:], in_=t['t_x'].ap().rearrange("c p f -> p c f"))
            cb = pyr.tile([P, 2], F32)
            nc.sync.dma_start(out=cb[:], in_=t['t_cb'].ap())
            nb = pyr.tile([P, 4, 2], F32)
            nc.sync.dma_start(out=nb[:], in_=t['t_nb'].ap().rearrange("l p f -> p l f"))

            # mem = relu(1x1 conv), fc processed in pairs (2 psum slots)
            for oc in range(2):
                for fp in range(2):
                    pss = [pms.tile([P, 512], F32, tag="m", name=f"mempp{oc}_{fp}_{i2}") for i2 in range(2)]
                    for cc in range(4):
                        wsl = wslice(t['t_w1x1'][cc, :, 128 * oc:128 * (oc + 1)])
                        for i, fc in enumerate((2 * fp, 2 * fp + 1)):
                            nc.tensor.matmul(pss[i][:], wsl[:],
                                             xT[:, cc, 1 + 512 * fc:1 + 512 * (fc + 1)],
                                             start=(cc == 0), stop=(cc == 3))
                    for i, fc in enumerate((2 * fp, 2 * fp + 1)):
                        nc.vector.tensor_scalar(out=memT[:, oc, 512 * fc:512 * (fc + 1)],
                                                in0=pss[i][:], scalar1=cb[:, oc:oc + 1],
                                                scalar2=0.0, op0=OP.add, op1=OP.max)

            # neck pyramid (feature-major, 1-col zero pad left)
            lvl_len = [1024, 512, 256, 128]
            lbufs = []
            src_buf = None
            for lv in range(4):
                L = lvl_len[lv]
                lb = pyr.tile([P, 2, L + 8], BF16, tag=f"lb{lv}", name=f"lb{lv}")
                nc.gpsimd.memset(lb[:], 0.0)
                lbufs.append(lb)
                n_cc = 4 if lv == 0 else 2
                nfc = (L + 511) // 512
                for oc in range(2):
                    pss = [pms.tile([P, 512], F32, tag="m", name=f"cvp{lv}_{oc}_{i2}") for i2 in range(nfc)]
                    k = 0
                    for cc in range(n_cc):
                        for tap in range(3):
                            if lv == 0:
                                wsl = wslice(t['t_wn0'][tap, cc, :, 128 * oc:128 * (oc + 1)])
                            else:
                                wsl = wslice(t['t_wnk'][lv - 1, tap, cc, :, 128 * oc:128 * (oc + 1)])
                            for fc in range(nfc):
                                w = min(512, L - 512 * fc)
                                rhs = (stride2(xT, cc, 1024 * fc + tap, w) if lv == 0
                                       else stride2(src_buf, cc, 1024 * fc + tap, w))
                                nc.tensor.matmul(pss[fc][:, :w], wsl[:], rhs,
                                                 start=(k == 0), stop=(k == 3 * n_cc - 1))
                            k += 1
                    for fc in range(nfc):
                        w = min(512, L - 512 * fc)
                        nc.vector.tensor_scalar(out=lb[:, oc, 1 + 512 * fc:1 + 512 * fc + w],
                                                in0=pss[fc][:, :w], scalar1=nb[:, lv, oc:oc + 1],
                                                scalar2=0.0, op0=OP.add, op1=OP.max)
                src_buf = lb

            # transpose + selection, per feature chunk
            f0 = state.tile([P, 2, RW], BF16, tag="fT")
            blk_of = []
            for lv in range(4):
                for j in range(lvl_len[lv] // 128):
                    blk_of.append((lv, j))
            for dc in range(2):
                sel_ps = pms.tile([P, 512], F32, tag="m")
                for b, (lv, j) in enumerate(blk_of):
                    tr_ps = pms.tile([P, P], BF16, tag="m")
                    nc.tensor.transpose(tr_ps[:],
                                        lbufs[lv][:, dc, 1 + 128 * j:1 + 128 * (j + 1)],
                                        ident_b[:])
                    fr = selw.tile([P, P], BF16, tag="frow")
                    nc.vector.tensor_copy(fr[:], tr_ps[:])
                    sb = selw.tile([P, RW], BF16, tag="ssel")
                    nc.sync.dma_start(out=sb[:], in_=t['t_ssel'][b])
                    nc.tensor.matmul(sel_ps[:], fr[:], sb[:],
                                     start=(b == 0), stop=(b == NBLK - 1))
                nc.vector.scalar_tensor_tensor(out=f0[:, dc, :], in0=sel_ps[:],
                                               scalar=float(np.sqrt(COUT)),
                                               in1=pe_sb[:, dc, :],
                                               op0=OP.mult, op1=OP.add)

        # ================= decoder layers =================
        import os as _os
        _nlayers = int(_os.environ.get('KERN_NLAYERS', str(NLY)))
        _stage = _os.environ.get('KERN_STAGE', 'all')
        fT = f0
        for l in range(_nlayers):
            b_sa = wb.tile([P, 6], F32, tag="b_sa")
            nc.sync.dma_start(out=b_sa[:], in_=t['t_sa_b'][l])
            b_sao = wb.tile([P, 2], F32, tag="b_sao")
            nc.sync.dma_start(out=b_sao[:], in_=t['t_sa_ob'][l])
            b_caq = wb.tile([P, 2], F32, tag="b_caq")
            nc.sync.dma_start(out=b_caq[:], in_=t['t_ca_qb'][l])
            b_cak = wb.tile([P, 2], F32, tag="b_cak")
            nc.sync.dma_start(out=b_cak[:], in_=t['t_ca_kb'][l])
            b_cao = wb.tile([P, 2], F32, tag="b_cao")
            nc.sync.dma_start(out=b_cao[:], in_=t['t_ca_ob'][l])
            b_ff1 = wb.tile([P, 8], F32, tag="b_ff1")
            nc.sync.dma_start(out=b_ff1[:], in_=t['t_ff1b'][l])
            b_ff2 = wb.tile([P, 2], F32, tag="b_ff2")
            nc.sync.dma_start(out=b_ff2[:], in_=t['t_ff2b'][l])
            gp = wb.tile([P, 6, 2], F32, tag="gp")
            nc.sync.dma_start(out=gp[:], in_=t['t_lngp'][l])

            # ---- cross K / V (independent of f) ----
            KT = kvp.tile([P, 2, T], BF16, tag="KT")
            for oc in range(2):
                wk = wslice(t['t_ca_kw'][l, :, :, 128 * oc:128 * (oc + 1)])
                for tp2 in range(2):
                    ps = psc.tile([P, 1024], F32, tag="sc")
                    for half in range(2):
                        tck = 2 * tp2 + half
                        for ic in range(2):
                            nc.tensor.matmul(ps[:, 512 * half:512 * (half + 1)],
                                             wk[:, ic, :],
                                             memT[:, ic, 512 * tck:512 * (tck + 1)],
                                             start=(ic == 0), stop=(ic == 1))
                    nc.vector.tensor_scalar_add(KT[:, oc, 1024 * tp2:1024 * (tp2 + 1)],
                                                ps[:], b_cak[:, oc:oc + 1])
            Vp = kvp.tile([P, 16, 512], BF16, tag="Vp")
            Vp_v = Vp[:].rearrange("p k (h two j) -> p k h two j", h=8, two=2)
            nc.gpsimd.memset(Vp_v[:, :, :, 1, :], 1.0)
            wv = wb.tile([P, 2, 512], BF16, tag="wv")
            nc.sync.dma_start(out=wv[:], in_=t['t_ca_vw'][l].rearrange("c p f -> p c f"))
            for kc in range(16):
                ps = pms.tile([P, 512], F32, tag="m")
                for ic in range(2):
                    nc.tensor.matmul(ps[:], memT[:, ic, 128 * kc:128 * (kc + 1)],
                                     wv[:, ic, :], start=(ic == 0), stop=(ic == 1))
                ps_v = ps[:].rearrange("p (h two j) -> p h two j", h=8, two=2)
                nc.vector.tensor_copy(Vp_v[:, kc, :, 0, :], ps_v[:, :, 0, :])

            if _stage == 'kv':
                continue
            # ---- self attention ----
            QTs = act.tile([P, 2, RW], BF16, tag="QTs")
            KTs = act.tile([P, 2, RW], BF16, tag="KTs")
            VTs = act.tile([P, 2, RW], BF16, tag="VTs")
            for wi, dst in ((0, QTs), (1, KTs), (2, VTs)):
                for oc in range(2):
                    wsl = wslice(t['t_sa_w'][l, :, :, 256 * wi + 128 * oc:256 * wi + 128 * (oc + 1)])
                    ps = pms.tile([P, 512], F32, tag="m")
                    for ic in range(2):
                        nc.tensor.matmul(ps[:], wsl[:, ic, :], fT[:, ic, :],
                                         start=(ic == 0), stop=(ic == 1))
                    nc.vector.tensor_scalar_add(dst[:, oc, :], ps[:],
                                                b_sa[:, 2 * wi + oc:2 * wi + oc + 1])
            OsT = act.tile([P, 2, RW], BF16, tag="OT")
            for sti, (qs, ql, ks) in enumerate(SUBTILES):
                vst_ps = pms.tile([P, 256], BF16, tag="m")
                for hc in range(2):
                    nc.tensor.transpose(vst_ps[:, 128 * hc:128 * (hc + 1)],
                                        VTs[:, hc, ks:ks + 128], ident_b[:])
                vst = work.tile([P, 512], BF16, tag="vst")
                vst_v = vst[:].rearrange("p (h two j) -> p h two j", h=8, two=2)
                nc.gpsimd.memset(vst_v[:, :, 1, :], 1.0)
                nc.vector.tensor_copy(vst_v[:, :, 0, :], vst_ps[:])
                mskt = work.tile([P, 256], BF16, tag="mskt")
                nc.sync.dma_start(out=mskt[:], in_=t['t_smask'][sti])
                for p in range(4):
                    h0, h1 = 2 * p, 2 * p + 1
                    sps = psc.tile([P, 1024], F32, tag="sc")
                    for hi, hh in enumerate((h0, h1)):
                        nc.tensor.matmul(sps[:, 512 * hi:512 * hi + ql],
                                         KTs[32 * (hh % 4):32 * (hh % 4) + 32, hh // 4, ks:ks + 128],
                                         QTs[32 * (hh % 4):32 * (hh % 4) + 32, hh // 4, qs:qs + ql],
                                         start=True, stop=False,
                                         tile_position=(32 * (hh % 4), 0))
                    for hi in range(2):
                        nc.tensor.matmul(sps[:, 512 * hi:512 * hi + ql], ident_b[:],
                                         mskt[:, 128 * hi:128 * hi + ql],
                                         start=False, stop=True)
                    spv = sps[:].rearrange("p (b q) -> p b q", b=2)[:, :, 0:ql]
                    es = epool.tile([P, 256], BF16, tag="E")
                    esv = es[:].rearrange("p (b q) -> p b q", b=2)[:, :, 0:ql]
                    nc.scalar.activation(out=esv, in_=spv, func=AF.Exp)
                    avp = pav.tile([P, 512], F32, tag="av")
                    nc.tensor.matmul(avp[0:64, :ql], vst[:, 64 * h0:64 * h0 + 64],
                                     es[:, 0:ql], start=True, stop=True,
                                     tile_position=(0, 0))
                    nc.tensor.matmul(avp[64:128, :ql], vst[:, 64 * h1:64 * h1 + 64],
                                     es[:, 128:128 + ql], start=True, stop=True,
                                     tile_position=(0, 64))
                    zr = work.tile([P, 512], F32, tag="zr")
                    nc.vector.reciprocal(zr[:, :ql], avp[:, :ql])
                    nc.vector.tensor_mul(OsT[64 * (p % 2):64 * (p % 2) + 32, p // 2, qs:qs + ql],
                                         avp[0:32, :ql], zr[32:64, :ql])
                    nc.vector.tensor_mul(OsT[64 * (p % 2) + 32:64 * (p % 2) + 64, p // 2, qs:qs + ql],
                                         avp[64:96, :ql], zr[96:128, :ql])
            R1 = state.tile([P, 2, RW], BF16, tag="fT")
            for oc in range(2):
                wsl = wslice(t['t_sa_ow'][l, :, :, 128 * oc:128 * (oc + 1)])
                ps = pms.tile([P, 512], F32, tag="m")
                for ic in range(2):
                    nc.tensor.matmul(ps[:], wsl[:, ic, :], OsT[:, ic, :],
                                     start=(ic == 0), stop=(ic == 1))
                nc.vector.scalar_tensor_tensor(out=R1[:, oc, :], in0=ps[:],
                                               scalar=b_sao[:, oc:oc + 1], in1=fT[:, oc, :],
                                               op0=OP.add, op1=OP.add)
            f1 = state.tile([P, 2, RW], BF16, tag="fT")
            _layernorm(nc, tc, pms, work, stats, act, R1, f1, gp, 0, invn, ones_f, eps_t)

            if _stage == 'self':
                fT = f1
                continue
            # ---- cross attention ----
            QTc = act.tile([P, 2, RW], BF16, tag="QTc")
            for oc in range(2):
                wsl = wslice(t['t_ca_qw'][l, :, :, 128 * oc:128 * (oc + 1)])
                ps = pms.tile([P, 512], F32, tag="m")
                for ic in range(2):
                    nc.tensor.matmul(ps[:], wsl[:, ic, :], f1[:, ic, :],
                                     start=(ic == 0), stop=(ic == 1))
                nc.vector.tensor_scalar_add(QTc[:, oc, :], ps[:], b_caq[:, oc:oc + 1])
            OcT = act.tile([P, 2, RW], BF16, tag="OT")
            for p in range(4):
                h0, h1 = 2 * p, 2 * p + 1
                avp = pav.tile([P, 512], F32, tag="av")
                for kc in range(16):
                    scp = psc.tile([P, 1024], F32, tag="sc")
                    for hi, hh in enumerate((h0, h1)):
                        nc.tensor.matmul(scp[:, 512 * hi:512 * (hi + 1)],
                                         KT[32 * (hh % 4):32 * (hh % 4) + 32, hh // 4, 128 * kc:128 * (kc + 1)],
                                         QTc[32 * (hh % 4):32 * (hh % 4) + 32, hh // 4, :],
                                         start=True, stop=True, tile_position=(32 * (hh % 4), 0))
                    ec = epool.tile([P, 1024], BF16, tag="E")
                    nc.scalar.activation(out=ec[:], in_=scp[:], func=AF.Exp)
                    st, sp = (kc == 0), (kc == 15)
                    nc.tensor.matmul(avp[0:64, :], Vp[:, kc, 64 * h0:64 * h0 + 64],
                                     ec[:, 0:512], start=st, stop=sp, tile_position=(0, 0))
                    nc.tensor.matmul(avp[64:128, :], Vp[:, kc, 64 * h1:64 * h1 + 64],
                                     ec[:, 512:1024], start=st, stop=sp, tile_position=(0, 64))
                zr = work.tile([P, 512], F32, tag="zr")
                nc.vector.reciprocal(zr[:, :], avp[:, :])
                nc.vector.tensor_mul(OcT[64 * (p % 2):64 * (p % 2) + 32, p // 2, :],
                                     avp[0:32, :], zr[32:64, :])
                nc.vector.tensor_mul(OcT[64 * (p % 2) + 32:64 * (p % 2) + 64, p // 2, :],
                                     avp[64:96, :], zr[96:128, :])
            R2 = state.tile([P, 2, RW], BF16, tag="fT")
            for oc in range(2):
                wsl = wslice(t['t_ca_ow'][l, :, :, 128 * oc:128 * (oc + 1)])
                ps = pms.tile([P, 512], F32, tag="m")
                for ic in range(2):
                    nc.tensor.matmul(ps[:], wsl[:, ic, :], OcT[:, ic, :],
                                     start=(ic == 0), stop=(ic == 1))
                nc.vector.scalar_tensor_tensor(out=R2[:, oc, :], in0=ps[:],
                                               scalar=b_cao[:, oc:oc + 1], in1=f1[:, oc, :],
                                               op0=OP.add, op1=OP.add)
            f2 = state.tile([P, 2, RW], BF16, tag="fT")
            _layernorm(nc, tc, pms, work, stats, act, R2, f2, gp, 1, invn, ones_f, eps_t)

            if _stage == 'cross':
                fT = f2
                continue
            # ---- ffn (Hb in 2 halves of 4) ----
            ps_oc = [pms.tile([P, 512], F32, tag="m", name=f"ffp{l}_{i2}") for i2 in range(2)]
            for hf in range(2):
                Hb = big.tile([P, 4, RW], BF16, tag="Hb")
                for jj in range(4):
                    j = 4 * hf + jj
                    w1 = wslice(t['t_ff1w'][l, :, :, 128 * j:128 * (j + 1)])
                    ps = psc.tile([P, 1024], F32, tag="sc")
                    for ic in range(2):
                        nc.tensor.matmul(ps[:, 0:512], w1[:, ic, :], f2[:, ic, :],
                                         start=(ic == 0), stop=(ic == 1))
                    nc.vector.tensor_scalar(out=Hb[:, jj, :], in0=ps[:, 0:512],
                                            scalar1=b_ff1[:, j:j + 1], scalar2=0.0,
                                            op0=OP.add, op1=OP.max)
                for jj in range(4):
                    j = 4 * hf + jj
                    w2 = wslice(t['t_ff2w'][l, j], tag="w256")
                    for oc in range(2):
                        nc.tensor.matmul(ps_oc[oc][:], w2[:, 128 * oc:128 * (oc + 1)],
                                         Hb[:, jj, :], start=(j == 0), stop=(j == 7))
            R3 = state.tile([P, 2, RW], BF16, tag="fT")
            for oc in range(2):
                nc.vector.scalar_tensor_tensor(out=R3[:, oc, :], in0=ps_oc[oc][:],
                                               scalar=b_ff2[:, oc:oc + 1], in1=f2[:, oc, :],
                                               op0=OP.add, op1=OP.add)
            f3 = state.tile([P, 2, RW], BF16, tag="fT")
            _layernorm(nc, tc, pms, work, stats, act, R3, f3, gp, 2, invn, ones_f, eps_t)
            fT = f3

        outf = state.tile([P, 2, RW], F32, tag="outf")
        nc.vector.tensor_copy(outf[:], fT[:])
        nc.sync.dma_start(out=t['t_out'].ap(), in_=outf[:])


def _layernorm(nc, tc, pms, work, stats, act, R, out, gp, which, invn, ones_f, eps_t):
    """Feature-major LN over d=256 (2 partition chunks), rows on free dim.
    Stats via all-(1/256) stationary matmuls producing 128-row broadcasts.
    gamma/beta are per-partition scalars (gp packs both: col 2*which=gamma,
    2*which+1=beta)."""
    P = 128
    sq = act.tile([P, 2, RW], BF16, tag="sq")
    nc.vector.tensor_mul(sq[:, 0, :], R[:, 0, :], R[:, 0, :])
    nc.vector.tensor_mul(sq[:, 1, :], R[:, 1, :], R[:, 1, :])
    mB = pms.tile([P, 512], F32, tag="m")
    for ic in range(2):
        nc.tensor.matmul(mB[:], invn[:], R[:, ic, :], start=(ic == 0), stop=(ic == 1))
    msB = pms.tile([P, 512], F32, tag="m")
    for ic in range(2):
        nc.tensor.matmul(msB[:], invn[:], sq[:, ic, :], start=(ic == 0), stop=(ic == 1))
    m2 = stats.tile([1, RW], F32, tag="s1")
    nc.scalar.activation(out=m2[:], in_=mB[0:1, :], func=AF.Square)
    var = stats.tile([1, RW], F32, tag="s2")
    nc.vector.scalar_tensor_tensor(out=var[:], in0=msB[0:1, :], scalar=1.0,
                                   in1=m2[:], op0=OP.mult, op1=OP.subtract)
    sd = stats.tile([1, RW], F32, tag="s1")
    nc.scalar.activation(out=sd[:], in_=var[:], func=AF.Sqrt, bias=eps_t[:])
    r = stats.tile([1, RW], F32, tag="s2")
    nc.vector.reciprocal(r[:], sd[:])
    rb = stats.tile([1, RW], BF16, tag="s3")
    nc.vector.tensor_copy(rb[:], r[:])
    rB = pms.tile([P, 512], F32, tag="m")
    nc.tensor.matmul(rB[:], ones_f[:], rb[:], start=True, stop=True)
    for oc in range(2):
        # c = (R - mB) * rB ; out = c * gamma + beta   (gamma/beta per-partition)
        c = work.tile([P, RW], F32, tag="tmp")
        nc.vector.tensor_sub(c[:], R[:, oc, :], mB[:])
        d = work.tile([P, RW], F32, tag="tmp2")
        nc.vector.tensor_mul(d[:], c[:], rB[:])
        nc.vector.tensor_scalar(out=out[:, oc, :], in0=d[:],
                                scalar1=gp[:, 2 * which, oc:oc + 1],
                                scalar2=gp[:, 2 * which + 1, oc:oc + 1],
                                op0=OP.mult, op1=OP.add)


# ---------------------------------------------------------------------------
# host side
# ---------------------------------------------------------------------------

def _sinusoidal_pe(t, d):
    pos = np.arange(t, dtype=np.float32)[:, None]
    div = np.exp(np.arange(0, d, 2, dtype=np.float32) * (-np.log(10000.0) / d))
    ang = pos * div
    pe = np.zeros((t, d), np.float32)
    pe[:, 0::2] = np.sin(ang)
    pe[:, 1::2] = np.cos(ang)
    return pe


def _concat_row_to_level(r):
    for li in range(NLV):
        if r < LVL_STARTS[li] + LVL_SIZES[li]:
            return li, r - LVL_STARTS[li]
    raise ValueError(r)


def _core_meta(c):
    w0 = OWN * c - HALO
    S = np.zeros((TOT, RW), np.float32)
    valid = np.zeros(RW, bool)
    lvl_of = np.full(RW, -1)
    pos_of = np.full(RW, -1)
    for j in range(RW):
        r = w0 + j
        if 0 <= r < TOT:
            S[r, j] = 1.0
            valid[j] = True
            lvl_of[j], pos_of[j] = _concat_row_to_level(r)
    pes = [_sinusoidal_pe(sz, COUT) for sz in LVL_SIZES]
    pe_plus = np.zeros((COUT, RW), np.float32)
    for j in range(RW):
        if valid[j]:
            pe_plus[:, j] = pes[lvl_of[j]][pos_of[j]]
    smask = np.full((5, 128, 256), -1e9, np.float32)
    for sti, (qs, ql, ks) in enumerate(SUBTILES):
        m = np.full((128, ql), -1e9, np.float32)
        for jq in range(ql):
            q = qs + jq
            for jk in range(128):
                k = ks + jk
                if k >= RW:
                    continue
                if valid[q] and valid[k]:
                    if lvl_of[q] == lvl_of[k] and abs(pos_of[q] - pos_of[k]) <= KBAND // 2:
                        m[jk, jq] = 0.0
                elif (not valid[q]) and k == q:
                    m[jk, jq] = 0.0
        smask[sti, :, 0:ql] = m
        smask[sti, :, 128:128 + ql] = m
    return S, pe_plus, smask


def _chunk_p(v):
    """[n*128] -> [128, n] partition-major."""
    v = np.asarray(v, np.float32)
    n = v.shape[0] // 128
    return v.reshape(n, 128).T.copy()


def _lhsT(w):
    """[O, I] weight -> [n_ic, 128, O] lhsT chunks (W^T chunked over I)."""
    wT = np.ascontiguousarray(np.asarray(w, np.float32).T)  # [I, O]
    I = wT.shape[0]
    return wT.reshape(I // 128, 128, wT.shape[1])


_NC_CACHE = None
LAST_EXEC_NS = None


def _get_nc():
    global _NC_CACHE
    if _NC_CACHE is None:
        _NC_CACHE = _build_nc()
    return _NC_CACHE


def _bf(a):
    return np.asarray(a, np.float32).astype(ml_dtypes.bfloat16)


def _prepare_in_maps(inputs):
    inp = {k: np.asarray(v, np.float32) for k, v in inputs.items()}

    scale = 1.0 / np.sqrt(HD)
    common = {}
    common['w1x1'] = _bf(_lhsT(inp['conv_w'][:, :, 0]))
    common['wn0'] = _bf(np.stack([_lhsT(inp['neck_w0'][:, :, tp]) for tp in range(3)]))
    common['wnk'] = _bf(np.stack([np.stack([_lhsT(inp['neck_w'][lv][:, :, tp]) for tp in range(3)])
                                  for lv in range(3)]))
    common['cbias'] = _chunk_p(inp['conv_b'])
    common['nbias'] = np.stack([_chunk_p(inp['neck_b0'])] +
                               [_chunk_p(inp['neck_b'][i]) for i in range(3)])

    sa_w, sa_b = [], []
    for l in range(NLY):
        w = inp['sa_in_w'][l].copy()    # [768, 256]
        b = inp['sa_in_b'][l].copy()
        w[:COUT] *= scale
        b[:COUT] *= scale
        sa_w.append(_lhsT(w))           # [2, 128, 768]
        bb = np.zeros((128, 6), np.float32)
        for wi in range(3):
            bb[:, 2 * wi:2 * wi + 2] = _chunk_p(b[wi * COUT:(wi + 1) * COUT])
        sa_b.append(bb)
    common['sa_w'] = _bf(np.stack(sa_w))
    common['sa_b'] = np.stack(sa_b)
    common['sa_ow'] = _bf(np.stack([_lhsT(inp['sa_out_w'][l]) for l in range(NLY)]))
    common['sa_ob'] = np.stack([_chunk_p(inp['sa_out_b'][l]) for l in range(NLY)])
    common['ca_qw'] = _bf(np.stack([_lhsT(inp['ca_in_w'][l][:COUT] * scale) for l in range(NLY)]))
    common['ca_qb'] = np.stack([_chunk_p(inp['ca_in_b'][l][:COUT] * scale) for l in range(NLY)])
    common['ca_kw'] = _bf(np.stack([_lhsT(inp['ca_in_w'][l][COUT:2 * COUT]) for l in range(NLY)]))
    common['ca_kb'] = np.stack([_chunk_p(inp['ca_in_b'][l][COUT:2 * COUT]) for l in range(NLY)])
    ca_vw = []
    for l in range(NLY):
        wT = _lhsT(inp['ca_in_w'][l][2 * COUT:])          # [2, 128, 256]
        waug = np.zeros((2, 128, 512), np.float32)
        for hh2 in range(H):
            waug[:, :, 64 * hh2:64 * hh2 + 32] = wT[:, :, 32 * hh2:32 * hh2 + 32]
        ca_vw.append(waug)
    common['ca_vw'] = _bf(np.stack(ca_vw))
    common['ca_ow'] = _bf(np.stack([_lhsT(inp['ca_out_w'][l]) for l in range(NLY)]))
    # fold cross-attn V bias through the output projection into its bias
    common['ca_ob'] = np.stack([
        _chunk_p(inp['ca_out_b'][l] + inp['ca_out_w'][l] @ inp['ca_in_b'][l][2 * COUT:])
        for l in range(NLY)])
    common['ff1w'] = _bf(np.stack([_lhsT(inp['ff1_w'][l]) for l in range(NLY)]))
    common['ff1b'] = np.stack([_chunk_p(inp['ff1_b'][l]) for l in range(NLY)])
    common['ff2w'] = _bf(np.stack([_lhsT(inp['ff2_w'][l]) for l in range(NLY)]))
    common['ff2b'] = np.stack([_chunk_p(inp['ff2_b'][l]) for l in range(NLY)])
    lngp = np.zeros((NLY, 128, 6, 2), np.float32)
    for l in range(NLY):
        for wi, (g, b) in enumerate(((inp['ln1_g'][l], inp['ln1_b'][l]),
                                     (inp['ln2_g'][l], inp['ln2_b'][l]),
                                     (inp['ln3_g'][l], inp['ln3_b'][l]))):
            lngp[l, :, 2 * wi, :] = _chunk_p(g)
            lngp[l, :, 2 * wi + 1, :] = _chunk_p(b)
    common['lngp'] = lngp

    metas = [_core_meta(c) for c in range(4)]
    in_maps = []
    for core in range(8):
        b, c = core // 4, core % 4
        S, pe_plus, smask = metas[c]
        xp = np.zeros((CIN, XP), np.float32)
        xp[:, 1:1 + T] = inp['x'][b]
        m = dict(common)
        m['xp'] = _bf(xp.reshape(4, 128, XP))
        m['ssel'] = _bf(S.reshape(NBLK, 128, RW))
        m['pe'] = _bf(pe_plus.reshape(2, 128, RW).transpose(1, 0, 2))
        m['smask'] = _bf(smask)
        in_maps.append(m)
    return in_maps


def kernel(**inputs):
    nc = _get_nc()
    in_maps = _prepare_in_maps(inputs)

    global LAST_EXEC_NS
    trace = bool(int(os.environ.get('KERN_TRACE', '0')))
    res = run_bass_kernel_spmd(nc, in_maps, list(range(8)), trace=trace)
    if res.exec_time_ns is not None:
        LAST_EXEC_NS = res.exec_time_ns

    out = np.zeros((B, COUT, TOT), np.float32)
    for core in range(8):
        b, c = core // 4, core % 4
        o = res.results[core]['out']          # [128, 2, RW]
        fT = o.transpose(1, 0, 2).reshape(COUT, RW)
        out[b, :, OWN * c:OWN * (c + 1)] = fT[:, HALO:HALO + OWN]
    return out


def timeline_estimate():
    """Cost-model single-core timeline estimate (ns)."""
    from concourse.timeline_sim import TimelineSim
    nc = _get_nc()
    ts = TimelineSim(nc, trace=False)
    ts.simulate()
    return ts


# revision 36
# speedup vs baseline: 1.8231x; 1.1990x over previous
"""Trainium2 Bass kernel for nn_AttnFPN (conv pyramid + 4-layer transformer
decoder with banded self-attention + dense cross-attention over a conv memory).

Sharding: 8 cores = 2 batches x 4 window-quarters of the concatenated pyramid
row space (1920 rows). Each core computes the full conv pyramid for its batch,
selects a 512-row window (480 owned rows + 16-row halo each side) via a
one-hot selection matmul, runs all 4 decoder layers on the window (halo
shrink absorbs the banded self-attention's +-4 reach per layer), and emits its
480 owned rows. The host assembles the [B, 256, 1920] output.

On-chip layout is feature-major throughout: activations live as X^T
[d on partitions (2x128 chunks), rows on free dim]. All matmul operands are
bf16 (PE: 1 cycle/row vs 4 for fp32); PSUM accumulation stays fp32. Softmax
runs without max subtraction (scores empirically bounded; exp(-1e9)
underflows to 0 for the band mask). Softmax denominators are produced as
32-row broadcasts by interleaved all-ones columns in the V stationary tiles
(ones written by memset; cross-attn V bias is folded into the output-proj
bias on the host). LayerNorm stats use all-(1/256) stationary matmuls; the
rstd is exp(-0.5*ln(var+eps)) so the Act engine never leaves the exp table
set; LN betas are folded into downstream projection biases host-side.
"""
import os
import sys

for _p in ('/opt/trn_rl_repo', os.path.expanduser('~/.axon_site/_ro/trn_rl_repo')):
    if os.path.isdir(_p) and _p not in sys.path:
        sys.path.insert(0, _p)

import ml_dtypes
import numpy as np

import concourse.bass as bass
import concourse.mybir as mybir
import concourse.tile as tile
from concourse import bacc
from concourse.bass_utils import run_bass_kernel_spmd
from concourse.masks import make_identity

F32 = mybir.dt.float32
BF16 = mybir.dt.bfloat16
AF = mybir.ActivationFunctionType
OP = mybir.AluOpType

# problem constants
B, CIN, COUT, T, NLV, NLY, H, DFF, KBAND = 2, 512, 256, 2048, 4, 4, 8, 1024, 9
HD = COUT // H           # 32
RW = 512                 # per-core window rows
OWN = 480
HALO = 16
LVL_SIZES = [1024, 512, 256, 128]
LVL_STARTS = [0, 1024, 1536, 1792]
TOT = 1920
NBLK = TOT // 128        # 15 row-blocks of the concat pyramid
XP = 2056                # padded x length (col j holds x[:, j-1], col 0 = zero)
# self-attn subtiles: (q_start, q_len, k_start) window-local
SUBTILES = [(0, 120, 0), (120, 120, 116), (240, 120, 236), (360, 120, 356), (480, 32, 384)]

# packed per-layer bias/param columns (t_lb)
LB_SA, LB_SAO, LB_CAQ, LB_CAK, LB_CAO = 0, 6, 8, 10, 12
LB_FF1, LB_FF2, LB_G, LB_B3 = 14, 22, 24, 30
LB_W = 32

# ---------------------------------------------------------------------------
# device program
# ---------------------------------------------------------------------------


def _build_nc():
    nc = bacc.Bacc("TRN2", target_bir_lowering=False, debug=False, num_devices=8)

    def din(name, shape, dt=BF16):
        return nc.dram_tensor(name, list(shape), dt, kind="ExternalInput")

    t_x = din("xp", [4, 128, XP])                 # x padded, feature chunks
    t_ssel = din("ssel", [NBLK, 128, RW])         # one-hot selection
    t_pe = din("pe", [128, 2, RW])                # sqrt(C)*0 + PE slice, chunked
    t_smask = din("smask", [5, 128, 256])         # additive self masks per subtile
    # conv weights (lhsT layouts [i-chunk 128, o])
    t_w1x1 = din("w1x1", [4, 128, 256])
    t_wn0 = din("wn0", [3, 4, 128, 256])          # tap, cc, 128, 256
    t_wnk = din("wnk", [3, 3, 2, 128, 256])       # lvl-1, tap, cc, 128, 256
    t_cnb = din("cnb", [128, 10], F32)            # conv_b (2) + neck biases (4x2)
    # per-layer transformer weights
    t_sa_w = din("sa_w", [NLY, 2, 128, 768])      # qkv (q pre-scaled)
    t_sa_ow = din("sa_ow", [NLY, 2, 128, 256])
    t_ca_qw = din("ca_qw", [NLY, 2, 128, 256])    # pre-scaled
    t_ca_kw = din("ca_kw", [NLY, 2, 128, 256])
    t_ca_vw = din("ca_vw", [NLY, 2, 128, 512])    # head-interleaved, ones cols zero
    t_ca_ow = din("ca_ow", [NLY, 2, 128, 256])
    t_ff1w = din("ff1w", [NLY, 2, 128, 1024])
    t_ff2w = din("ff2w", [NLY, 8, 128, 256])
    t_lb = din("lb", [NLY, 128, LB_W], F32)       # packed biases/gammas (betas folded)
    t_out = nc.dram_tensor("out", [128, 2, RW], F32, kind="ExternalOutput")

    with tile.TileContext(nc) as tc:
        _emit(nc, tc, locals())
    nc.compile()
    return nc


def _emit(nc, tc, t):
    from contextlib import ExitStack
    ctx = ExitStack()
    with ctx:
        P = 128
        persist = ctx.enter_context(tc.tile_pool(name="persist", bufs=1))
        state = ctx.enter_context(tc.tile_pool(name="state", bufs=5))
        big = ctx.enter_context(tc.tile_pool(name="big", bufs=1))
        kvp = ctx.enter_context(tc.tile_pool(name="kvp", bufs=2))
        wroll = ctx.enter_context(tc.tile_pool(name="wroll", bufs=2))
        wb = ctx.enter_context(tc.tile_pool(name="wb", bufs=2))
        work = ctx.enter_context(tc.tile_pool(name="work", bufs=2))
        stats = ctx.enter_context(tc.tile_pool(name="stats", bufs=1))
        act = ctx.enter_context(tc.tile_pool(name="act", bufs=1))
        epool = ctx.enter_context(tc.tile_pool(name="epool", bufs=5))
        psc = ctx.enter_context(tc.tile_pool(name="psc", bufs=2, space="PSUM"))
        pav = ctx.enter_context(tc.tile_pool(name="pav", bufs=2, space="PSUM"))
        pms = ctx.enter_context(tc.tile_pool(name="pms", bufs=2, space="PSUM"))

        def stride2(ap3, cc, s, w):
            return ap3[:, cc, s:s + 2 * w].rearrange("p (n two) -> p two n", two=2)[:, 0, :]

        # ---- constants ----
        ident = persist.tile([P, P], F32)
        make_identity(nc, ident[:])
        ident_b = persist.tile([P, P], BF16)
        nc.vector.tensor_copy(ident_b[:], ident[:])
        invn = persist.tile([P, P], BF16)
        nc.gpsimd.memset(invn[:], 1.0 / COUT)
        eps_t = persist.tile([P, 1], F32)
        nc.gpsimd.memset(eps_t[:], 1e-5)

        pe_sb = persist.tile([P, 2, RW], BF16)
        nc.sync.dma_start(out=pe_sb[:], in_=t['t_pe'].ap())
        smask_sb = persist.tile([P, 5, 256], BF16)
        nc.sync.dma_start(out=smask_sb[:], in_=t['t_smask'].ap().rearrange("s p f -> p s f"))

        memT = persist.tile([P, 2, T], BF16)

        import os as _os
        _nlayers = int(_os.environ.get('KERN_NLAYERS', str(NLY)))
        _stage = _os.environ.get('KERN_STAGE', 'all')

        def kv_steps(l, lb_l):
            """Yield KV-projection work for layer l in small chunks so it can
            be interleaved under Act-bound phases. First yields (KT, Vp)."""
            KT = kvp.tile([P, 2, T], BF16, tag="KT", name=f"KT{l}")
            Vp = kvp.tile([P, 16, 512], BF16, tag="Vp", name=f"Vp{l}")
            wk = wroll.tile([P, 2, 256], BF16, tag="wcak")
            nc.sync.dma_start(out=wk[:], in_=t['t_ca_kw'][l].rearrange("c p f -> p c f"))
            wv = wroll.tile([P, 2, 512], BF16, tag="wv")
            nc.sync.dma_start(out=wv[:], in_=t['t_ca_vw'][l].rearrange("c p f -> p c f"))
            Vp_v = Vp[:].rearrange("p k (h two j) -> p k h two j", h=8, two=2)
            nc.gpsimd.memset(Vp_v[:, :, :, 1, :], 1.0)
            yield (KT, Vp)
            for oc in range(2):
                for q in range(4):
                    ps = pms.tile([P, 512], F32, tag="m")
                    for ic in range(2):
                        nc.tensor.matmul(ps[:], wk[:, ic, 128 * oc:128 * (oc + 1)],
                                         memT[:, ic, 512 * q:512 * (q + 1)],
                                         start=(ic == 0), stop=(ic == 1))
                    nc.vector.tensor_scalar_add(KT[:, oc, 512 * q:512 * (q + 1)],
                                                ps[:], lb_l[:, LB_CAK + oc:LB_CAK + oc + 1])
                    yield None
            for kc in range(16):
                ps = pms.tile([P, 512], F32, tag="m")
                for ic in range(2):
                    nc.tensor.matmul(ps[:], memT[:, ic, 128 * kc:128 * (kc + 1)],
                                     wv[:, ic, :], start=(ic == 0), stop=(ic == 1))
                ps_v = ps[:].rearrange("p (h two j) -> p h two j", h=8, two=2)
                nc.vector.tensor_copy(Vp_v[:, kc, :, 0, :], ps_v[:, :, 0, :])
                if kc % 2 == 1:
                    yield None

        def drain(gen):
            if gen is not None:
                for _ in gen:
                    pass

        lbs_t = []
        for l in range(_nlayers):
            lb_l = wb.tile([P, LB_W], F32, tag="lb", name=f"lb{l}")
            nc.sync.dma_start(out=lb_l[:], in_=t['t_lb'][l])
            lbs_t.append(lb_l)
        kv_gen = kv_steps(0, lbs_t[0]) if _nlayers else None
        kv_tiles = next(kv_gen) if kv_gen else None

        # ================= pyramid =================
        with tc.tile_pool(name="pyr", bufs=1) as pyr:
            xT = pyr.tile([P, 4, XP], BF16)
            for cc in range(4):
                nc.sync.dma_start(out=xT[:, cc, :], in_=t['t_x'][cc])
            cnb = pyr.tile([P, 10], F32)
            nc.sync.dma_start(out=cnb[:], in_=t['t_cnb'].ap())
            w1x1 = pyr.tile([P, 4, 256], BF16)
            nc.sync.dma_start(out=w1x1[:], in_=t['t_w1x1'].ap().rearrange("c p f -> p c f"))
            wn0 = pyr.tile([P, 12, 256], BF16)
            nc.sync.dma_start(out=wn0[:], in_=t['t_wn0'].ap().rearrange("t c p f -> p (t c) f"))
            wnk = pyr.tile([P, 18, 256], BF16)
            nc.sync.dma_start(out=wnk[:], in_=t['t_wnk'].ap().rearrange("l t c p f -> p (l t c) f"))
            ssel = pyr.tile([P, NBLK, RW], BF16)
            nc.sync.dma_start(out=ssel[:], in_=t['t_ssel'].ap().rearrange("b p f -> p b f"))

            # mem = relu(1x1 conv), fc processed in pairs (2 psum slots)
            for oc in range(2):
                for fp in range(2):
                    pss = [pms.tile([P, 512], F32, tag="m", name=f"mempp{oc}_{fp}_{i2}") for i2 in range(2)]
                    for cc in range(4):
                        wsl = w1x1[:, cc, 128 * oc:128 * (oc + 1)]
                        for i, fc in enumerate((2 * fp, 2 * fp + 1)):
                            nc.tensor.matmul(pss[i][:], wsl,
                                             xT[:, cc, 1 + 512 * fc:1 + 512 * (fc + 1)],
                                             start=(cc == 0), stop=(cc == 3))
                    for i, fc in enumerate((2 * fp, 2 * fp + 1)):
                        nc.vector.tensor_scalar(out=memT[:, oc, 512 * fc:512 * (fc + 1)],
                                                in0=pss[i][:], scalar1=cnb[:, oc:oc + 1],
                                                scalar2=0.0, op0=OP.add, op1=OP.max)

            # neck pyramid (feature-major, 1-col zero pad left); each level's
            # transpose+selection blocks are emitted right after the level's
            # conv so they overlap the next level's conv (disjoint PSUM tags:
            # conv='m', transposes='sc', selection accumulators='av')
            lvl_len = [1024, 512, 256, 128]
            sel_ps = [pav.tile([P, 512], F32, tag="av", name=f"selps{dc}") for dc in range(2)]
            f0 = state.tile([P, 2, RW], BF16, tag="fT")
            blk0 = [0, 8, 12, 14]
            src_buf = None
            for lv in range(4):
                L = lvl_len[lv]
                lb_t = pyr.tile([P, 2, L + 8], BF16, tag=f"lb{lv}", name=f"lb{lv}")
                nc.gpsimd.memset(lb_t[:], 0.0)
                n_cc = 4 if lv == 0 else 2
                nfc = (L + 511) // 512
                for oc in range(2):
                    pss = [pms.tile([P, 512], F32, tag="m", name=f"cvp{lv}_{oc}_{i2}") for i2 in range(nfc)]
                    k = 0
                    for cc in range(n_cc):
                        for tap in range(3):
                            if lv == 0:
                                wsl = wn0[:, 4 * tap + cc, 128 * oc:128 * (oc + 1)]
                            else:
                                wsl = wnk[:, 6 * (lv - 1) + 2 * tap + cc, 128 * oc:128 * (oc + 1)]
                            for fc in range(nfc):
                                w = min(512, L - 512 * fc)
                                rhs = (stride2(xT, cc, 1024 * fc + tap, w) if lv == 0
                                       else stride2(src_buf, cc, 1024 * fc + tap, w))
                                nc.tensor.matmul(pss[fc][:, :w], wsl, rhs,
                                                 start=(k == 0), stop=(k == 3 * n_cc - 1))
                            k += 1
                    for fc in range(nfc):
                        w = min(512, L - 512 * fc)
                        nc.vector.tensor_scalar(out=lb_t[:, oc, 1 + 512 * fc:1 + 512 * fc + w],
                                                in0=pss[fc][:, :w], scalar1=cnb[:, 2 + 2 * lv + oc:3 + 2 * lv + oc],
                                                scalar2=0.0, op0=OP.add, op1=OP.max)
                src_buf = lb_t
                for j in range(L // 128):
                    b = blk0[lv] + j
                    for dc in range(2):
                        tr_ps = psc.tile([P, P], BF16, tag="sc")
                        nc.tensor.transpose(tr_ps[:],
                                            lb_t[:, dc, 1 + 128 * j:1 + 128 * (j + 1)],
                                            ident_b[:])
                        fr = work.tile([P, P], BF16, tag="frow")
                        nc.vector.tensor_copy(fr[:], tr_ps[:])
                        nc.tensor.matmul(sel_ps[dc][:], fr[:], ssel[:, b, :],
                                         start=(b == 0), stop=(b == NBLK - 1))
                    if kv_gen is not None:
                        next(kv_gen, None)
            drain(kv_gen)
            kv_gen = None
            for dc in range(2):
                nc.vector.scalar_tensor_tensor(out=f0[:, dc, :], in0=sel_ps[dc][:],
                                               scalar=float(np.sqrt(COUT)),
                                               in1=pe_sb[:, dc, :],
                                               op0=OP.mult, op1=OP.add)

        # ================= decoder layers =================
        fT = f0

        def _unused_kv_steps(l, lb_l):
            """Yield KV-projection work for layer l in small chunks so it can
            be interleaved under the Act-bound cross-attn of layer l-1.
            Yields after every few PE ops; first yields (KT, Vp) tiles."""
            KT = kvp.tile([P, 2, T], BF16, tag="KT", name=f"KT{l}")
            Vp = kvp.tile([P, 16, 512], BF16, tag="Vp", name=f"Vp{l}")
            wk = wroll.tile([P, 2, 256], BF16, tag="wcak")
            nc.sync.dma_start(out=wk[:], in_=t['t_ca_kw'][l].rearrange("c p f -> p c f"))
            wv = wroll.tile([P, 2, 512], BF16, tag="wv")
            nc.sync.dma_start(out=wv[:], in_=t['t_ca_vw'][l].rearrange("c p f -> p c f"))
            Vp_v = Vp[:].rearrange("p k (h two j) -> p k h two j", h=8, two=2)
            nc.gpsimd.memset(Vp_v[:, :, :, 1, :], 1.0)
            yield (KT, Vp)
            for oc in range(2):
                for q in range(4):
                    ps = pms.tile([P, 512], F32, tag="m")
                    for ic in range(2):
                        nc.tensor.matmul(ps[:], wk[:, ic, 128 * oc:128 * (oc + 1)],
                                         memT[:, ic, 512 * q:512 * (q + 1)],
                                         start=(ic == 0), stop=(ic == 1))
                    nc.vector.tensor_scalar_add(KT[:, oc, 512 * q:512 * (q + 1)],
                                                ps[:], lb_l[:, LB_CAK + oc:LB_CAK + oc + 1])
                    yield None
            for kc in range(16):
                ps = pms.tile([P, 512], F32, tag="m")
                for ic in range(2):
                    nc.tensor.matmul(ps[:], memT[:, ic, 128 * kc:128 * (kc + 1)],
                                     wv[:, ic, :], start=(ic == 0), stop=(ic == 1))
                ps_v = ps[:].rearrange("p (h two j) -> p h two j", h=8, two=2)
                nc.vector.tensor_copy(Vp_v[:, kc, :, 0, :], ps_v[:, :, 0, :])
                if kc % 2 == 1:
                    yield None

        def drain(gen):
            if gen is not None:
                for _ in gen:
                    pass

        for l in range(_nlayers):
            lb = lbs_t[l]
            KT, Vp = kv_tiles
            if l + 1 < _nlayers:
                kv_gen = kv_steps(l + 1, lbs_t[l + 1])
                kv_tiles = next(kv_gen)
            else:
                kv_gen = None

            if _stage == 'kv':
                drain(kv_gen)
                continue
            # ---- self attention ----
            wsa = wroll.tile([P, 2, 768], BF16, tag="wsa")
            nc.sync.dma_start(out=wsa[:], in_=t['t_sa_w'][l].rearrange("c p f -> p c f"))
            QTs = act.tile([P, 2, RW], BF16, tag="QTs")
            KTs = act.tile([P, 2, RW], BF16, tag="KTs")
            VTs = act.tile([P, 2, RW], BF16, tag="VTs")
            for wi, dst in ((0, QTs), (1, KTs), (2, VTs)):
                for oc in range(2):
                    ps = pms.tile([P, 512], F32, tag="m")
                    for ic in range(2):
                        nc.tensor.matmul(ps[:], wsa[:, ic, 256 * wi + 128 * oc:256 * wi + 128 * (oc + 1)],
                                         fT[:, ic, :],
                                         start=(ic == 0), stop=(ic == 1))
                    nc.vector.tensor_scalar_add(dst[:, oc, :], ps[:],
                                                lb[:, LB_SA + 2 * wi + oc:LB_SA + 2 * wi + oc + 1])
            OsT = act.tile([P, 2, RW], BF16, tag="OT")
            for sti, (qs, ql, ks) in enumerate(SUBTILES):
                # vst: transposed V block [keys, 8*32 vdims] + 32 ones cols
                vst = work.tile([P, 288], BF16, tag="vst")
                for hc in range(2):
                    vst_ps = pms.tile([P, 128], BF16, tag="m")
                    nc.tensor.transpose(vst_ps[:],
                                        VTs[:, hc, ks:ks + 128], ident_b[:])
                    nc.vector.tensor_copy(vst[:, 128 * hc:128 * (hc + 1)], vst_ps[:])
                nc.gpsimd.memset(vst[:, 256:288], 1.0)
                # all 8 heads' scores (+mask) in one PSUM tile, one exp
                sps = psc.tile([P, 1024], F32, tag="sc")
                for hh in range(8):
                    nc.tensor.matmul(sps[:, 128 * hh:128 * hh + ql],
                                     KTs[32 * (hh % 4):32 * (hh % 4) + 32, hh // 4, ks:ks + 128],
                                     QTs[32 * (hh % 4):32 * (hh % 4) + 32, hh // 4, qs:qs + ql],
                                     start=True, stop=False,
                                     tile_position=(32 * (hh % 4), 0))
                    nc.tensor.matmul(sps[:, 128 * hh:128 * hh + ql], ident_b[:],
                                     smask_sb[:, sti, 0:ql],
                                     start=False, stop=True)
                es = epool.tile([P, 1024], BF16, tag="E")
                nc.scalar.activation(out=es[:].rearrange("p (h q) -> p h q", h=8)[:, :, 0:ql],
                                     in_=sps[:].rearrange("p (h q) -> p h q", h=8)[:, :, 0:ql],
                                     func=AF.Exp)
                # avp: rows 0:64 = AV (2 heads), 64:128 = denominators; cols 128p
                avp = pav.tile([P, 512], F32, tag="av")
                for p in range(4):
                    h0, h1 = 2 * p, 2 * p + 1
                    nc.tensor.matmul(avp[0:32, 128 * p:128 * p + ql], vst[:, 64 * p:64 * p + 32],
                                     es[:, 128 * h0:128 * h0 + ql], start=True, stop=True,
                                     tile_position=(0, 0))
                    nc.tensor.matmul(avp[32:64, 128 * p:128 * p + ql], vst[:, 64 * p + 32:64 * p + 64],
                                     es[:, 128 * h1:128 * h1 + ql], start=True, stop=True,
                                     tile_position=(0, 32))
                    nc.tensor.matmul(avp[64:96, 128 * p:128 * p + ql], vst[:, 256:288],
                                     es[:, 128 * h0:128 * h0 + ql], start=True, stop=True,
                                     tile_position=(0, 64))
                    nc.tensor.matmul(avp[96:128, 128 * p:128 * p + ql], vst[:, 256:288],
                                     es[:, 128 * h1:128 * h1 + ql], start=True, stop=True,
                                     tile_position=(0, 96))
                zr = work.tile([P, 512], F32, tag="zr")
                nc.vector.reciprocal(zr[64:128, :], avp[64:128, :])
                for p in range(4):
                    nc.vector.tensor_mul(OsT[64 * (p % 2):64 * (p % 2) + 64, p // 2, qs:qs + ql],
                                         avp[0:64, 128 * p:128 * p + ql],
                                         zr[64:128, 128 * p:128 * p + ql])
            R1 = state.tile([P, 2, RW], BF16, tag="fT")
            wsao = wroll.tile([P, 2, 256], BF16, tag="wsao")
            nc.sync.dma_start(out=wsao[:], in_=t['t_sa_ow'][l].rearrange("c p f -> p c f"))
            for oc in range(2):
                ps = pms.tile([P, 512], F32, tag="m")
                for ic in range(2):
                    nc.tensor.matmul(ps[:], wsao[:, ic, 128 * oc:128 * (oc + 1)], OsT[:, ic, :],
                                     start=(ic == 0), stop=(ic == 1))
                nc.vector.scalar_tensor_tensor(out=R1[:, oc, :], in0=ps[:],
                                               scalar=lb[:, LB_SAO + oc:LB_SAO + oc + 1], in1=fT[:, oc, :],
                                               op0=OP.add, op1=OP.add)
            if kv_gen is not None:
                for _ in range(3):
                    next(kv_gen, None)
            f1 = state.tile([P, 2, RW], BF16, tag="fT")
            _layernorm(nc, pms, work, stats, act, R1, f1, lb, 0, invn, eps_t)

            if _stage == 'self':
                drain(kv_gen)
                fT = f1
                continue
            # ---- cross attention: two q-halves, 4-head quads per exp
            # slice; the A-half post-chain (R2/LN2/FFN/LN3 cols 0:256) is
            # pumped under the B-half's Act-bound window ----
            wcaq = wroll.tile([P, 2, 256], BF16, tag="wcaq")
            nc.sync.dma_start(out=wcaq[:], in_=t['t_ca_qw'][l].rearrange("c p f -> p c f"))
            wcao = wroll.tile([P, 2, 256], BF16, tag="wcao")
            nc.sync.dma_start(out=wcao[:], in_=t['t_ca_ow'][l].rearrange("c p f -> p c f"))
            wf1 = wroll.tile([P, 2, 1024], BF16, tag="wf1")
            nc.sync.dma_start(out=wf1[:], in_=t['t_ff1w'][l].rearrange("c p f -> p c f"))
            wf2 = wroll.tile([P, 8, 256], BF16, tag="wf2")
            nc.sync.dma_start(out=wf2[:], in_=t['t_ff2w'][l].rearrange("c p f -> p c f"))
            QTc = act.tile([P, 2, RW], BF16, tag="QTc")
            for oc in range(2):
                ps = pms.tile([P, 512], F32, tag="m")
                for ic in range(2):
                    nc.tensor.matmul(ps[:], wcaq[:, ic, 128 * oc:128 * (oc + 1)], f1[:, ic, :],
                                     start=(ic == 0), stop=(ic == 1))
                nc.vector.tensor_scalar_add(QTc[:, oc, :], ps[:], lb[:, LB_CAQ + oc:LB_CAQ + oc + 1])
            OcT = act.tile([P, 2, RW], BF16, tag="OT")
            R2 = state.tile([P, 2, RW], BF16, tag="fT")
            f2 = state.tile([P, 2, RW], BF16, tag="fT")
            R3 = state.tile([P, 2, RW], BF16, tag="fT")
            f3 = state.tile([P, 2, RW], BF16, tag="fT")

            def emit_r2(c0, c1):
                w = c1 - c0
                for oc in range(2):
                    ps = pms.tile([P, 512], F32, tag="m")
                    for ic in range(2):
                        nc.tensor.matmul(ps[:, 0:w], wcao[:, ic, 128 * oc:128 * (oc + 1)],
                                         OcT[:, ic, c0:c1], start=(ic == 0), stop=(ic == 1))
                    nc.vector.scalar_tensor_tensor(out=R2[:, oc, c0:c1], in0=ps[:, 0:w],
                                                   scalar=lb[:, LB_CAO + oc:LB_CAO + oc + 1],
                                                   in1=f1[:, oc, c0:c1], op0=OP.add, op1=OP.add)

            def ffn_steps(c0, c1):
                """FFN1 -> FFN2 -> R3 for columns [c0, c1). No Act-engine ops,
                so it can pump under cross-attn without table switches."""
                w = c1 - c0
                ps_oc = [pms.tile([P, 512], F32, tag="m", name=f"ffp{l}_{i2}") for i2 in range(2)]
                for hf in range(2):
                    Hb = big.tile([P, 4, 512], BF16, tag="Hb")
                    for jj in range(4):
                        j = 4 * hf + jj
                        ps = psc.tile([P, 1024], F32, tag="sc")
                        for ic in range(2):
                            nc.tensor.matmul(ps[:, 0:w], wf1[:, ic, 128 * j:128 * (j + 1)],
                                             f2[:, ic, c0:c1], start=(ic == 0), stop=(ic == 1))
                        nc.vector.tensor_scalar(out=Hb[:, jj, 0:w], in0=ps[:, 0:w],
                                                scalar1=lb[:, LB_FF1 + j:LB_FF1 + j + 1], scalar2=0.0,
                                                op0=OP.add, op1=OP.max)
                        yield None
                    for jj in range(4):
                        j = 4 * hf + jj
                        for oc in range(2):
                            nc.tensor.matmul(ps_oc[oc][:, 0:w], wf2[:, j, 128 * oc:128 * (oc + 1)],
                                             Hb[:, jj, 0:w], start=(j == 0), stop=(j == 7))
                        yield None
                for oc in range(2):
                    nc.vector.scalar_tensor_tensor(out=R3[:, oc, c0:c1], in0=ps_oc[oc][:, 0:w],
                                                   scalar=lb[:, LB_FF2 + oc:LB_FF2 + oc + 1],
                                                   in1=f2[:, oc, c0:c1], op0=OP.add, op1=OP.add)
                    yield None

            for p in range(4):
                h0, h1 = 2 * p, 2 * p + 1
                avp = pav.tile([P, 512], F32, tag="av")
                for kc in range(16):
                    scp = psc.tile([P, 1024], F32, tag="sc")
                    for hi, hh in enumerate((h0, h1)):
                        nc.tensor.matmul(scp[:, 512 * hi:512 * (hi + 1)],
                                         KT[32 * (hh % 4):32 * (hh % 4) + 32, hh // 4, 128 * kc:128 * (kc + 1)],
                                         QTc[32 * (hh % 4):32 * (hh % 4) + 32, hh // 4, :],
                                         start=True, stop=True, tile_position=(32 * (hh % 4), 0))
                    ec = epool.tile([P, 1024], BF16, tag="E")
                    nc.scalar.activation(out=ec[:], in_=scp[:], func=AF.Exp)
                    st, sp = (kc == 0), (kc == 15)
                    nc.tensor.matmul(avp[0:64, :], Vp[:, kc, 64 * h0:64 * h0 + 64],
                                     ec[:, 0:512], start=st, stop=sp, tile_position=(0, 0))
                    nc.tensor.matmul(avp[64:128, :], Vp[:, kc, 64 * h1:64 * h1 + 64],
                                     ec[:, 512:1024], start=st, stop=sp, tile_position=(0, 64))
                    if kv_gen is not None and kc % 4 == 3:
                        next(kv_gen, None)
                zr = work.tile([P, 512], F32, tag="zr")
                nc.vector.reciprocal(zr[:, :], avp[:, :])
                nc.vector.tensor_mul(OcT[64 * (p % 2):64 * (p % 2) + 32, p // 2, :],
                                     avp[0:32, :], zr[32:64, :])
                nc.vector.tensor_mul(OcT[64 * (p % 2) + 32:64 * (p % 2) + 64, p // 2, :],
                                     avp[64:96, :], zr[96:128, :])
            drain(kv_gen)
            emit_r2(0, RW)
            _layernorm(nc, pms, work, stats, act, R2, f2, lb, 1, invn, eps_t)
            for _ in ffn_steps(0, RW):
                pass
            _layernorm(nc, pms, work, stats, act, R3, f3, lb, 2, invn, eps_t)
            fT = f3

        # final: add back LN3 beta (unfolded) and emit fp32
        outf = big.tile([P, 2, RW], F32, tag="outf")
        for oc in range(2):
            nc.vector.tensor_scalar_add(outf[:, oc, :], fT[:, oc, :],
                                        lb[:, LB_B3 + oc:LB_B3 + oc + 1])
        nc.sync.dma_start(out=t['t_out'].ap(), in_=outf[:])


def _layernorm(nc, pms, work, stats, act, R, out, lb, which, invn, eps_t, c0=0, c1=RW):
    """Feature-major LN over d=256 (2 partition chunks), rows c0:c1 on the
    free dim. Stats via all-(1/256) stationary matmuls producing 128-row
    broadcasts; gamma per-partition; beta folded into downstream biases."""
    P = 128
    w = c1 - c0
    sq = act.tile([P, 2, RW], BF16, tag="sq")
    for oc in range(2):
        nc.vector.tensor_mul(sq[:, oc, c0:c1], R[:, oc, c0:c1], R[:, oc, c0:c1])
    mB = pms.tile([P, 512], F32, tag="m")
    for ic in range(2):
        nc.tensor.matmul(mB[:, 0:w], invn[:], R[:, ic, c0:c1], start=(ic == 0), stop=(ic == 1))
    msB = pms.tile([P, 512], F32, tag="m")
    for ic in range(2):
        nc.tensor.matmul(msB[:, 0:w], invn[:], sq[:, ic, c0:c1], start=(ic == 0), stop=(ic == 1))
    sqm = stats.tile([P, RW], F32, tag="s1")
    nc.scalar.activation(out=sqm[:, 0:w], in_=mB[:, 0:w], func=AF.Square)
    varB = stats.tile([P, RW], F32, tag="s2")
    nc.vector.tensor_sub(varB[:, 0:w], msB[:, 0:w], sqm[:, 0:w])
    sdB = stats.tile([P, RW], F32, tag="s1")
    nc.scalar.activation(out=sdB[:, 0:w], in_=varB[:, 0:w], func=AF.Sqrt, bias=eps_t[:])
    zrB = stats.tile([P, RW], F32, tag="s2")
    nc.vector.reciprocal(zrB[:, 0:w], sdB[:, 0:w])
    for oc in range(2):
        # out = ((R - mB) * gamma) * zrB   (gamma per-partition; beta folded)
        c = work.tile([P, RW], F32, tag="tmp")
        nc.vector.tensor_sub(c[:, 0:w], R[:, oc, c0:c1], mB[:, 0:w])
        nc.vector.scalar_tensor_tensor(out=out[:, oc, c0:c1], in0=c[:, 0:w],
                                       scalar=lb[:, LB_G + 2 * which + oc:LB_G + 2 * which + oc + 1],
                                       in1=zrB[:, 0:w], op0=OP.mult, op1=OP.mult)


# ---------------------------------------------------------------------------
# host side
# ---------------------------------------------------------------------------

def _sinusoidal_pe(t, d):
    pos = np.arange(t, dtype=np.float32)[:, None]
    div = np.exp(np.arange(0, d, 2, dtype=np.float32) * (-np.log(10000.0) / d))
    ang = pos * div
    pe = np.zeros((t, d), np.float32)
    pe[:, 0::2] = np.sin(ang)
    pe[:, 1::2] = np.cos(ang)
    return pe


def _concat_row_to_level(r):
    for li in range(NLV):
        if r < LVL_STARTS[li] + LVL_SIZES[li]:
            return li, r - LVL_STARTS[li]
    raise ValueError(r)


def _core_meta(c):
    w0 = OWN * c - HALO
    S = np.zeros((TOT, RW), np.float32)
    valid = np.zeros(RW, bool)
    lvl_of = np.full(RW, -1)
    pos_of = np.full(RW, -1)
    for j in range(RW):
        r = w0 + j
        if 0 <= r < TOT:
            S[r, j] = 1.0
            valid[j] = True
            lvl_of[j], pos_of[j] = _concat_row_to_level(r)
    pes = [_sinusoidal_pe(sz, COUT) for sz in LVL_SIZES]
    pe_plus = np.zeros((COUT, RW), np.float32)
    for j in range(RW):
        if valid[j]:
            pe_plus[:, j] = pes[lvl_of[j]][pos_of[j]]
    smask = np.full((5, 128, 256), -1e9, np.float32)
    for sti, (qs, ql, ks) in enumerate(SUBTILES):
        m = np.full((128, ql), -1e9, np.float32)
        for jq in range(ql):
            q = qs + jq
            for jk in range(128):
                k = ks + jk
                if k >= RW:
                    continue
                if valid[q] and valid[k]:
                    if lvl_of[q] == lvl_of[k] and abs(pos_of[q] - pos_of[k]) <= KBAND // 2:
                        m[jk, jq] = 0.0
                elif (not valid[q]) and k == q:
                    m[jk, jq] = 0.0
        smask[sti, :, 0:ql] = m
        smask[sti, :, 128:128 + ql] = m
    return S, pe_plus, smask


def _chunk_p(v):
    """[n*128] -> [128, n] partition-major."""
    v = np.asarray(v, np.float32)
    n = v.shape[0] // 128
    return v.reshape(n, 128).T.copy()


def _lhsT(w):
    """[O, I] weight -> [n_ic, 128, O] lhsT chunks (W^T chunked over I)."""
    wT = np.ascontiguousarray(np.asarray(w, np.float32).T)  # [I, O]
    I = wT.shape[0]
    return wT.reshape(I // 128, 128, wT.shape[1])


_NC_CACHE = None
LAST_EXEC_NS = None


def _get_nc():
    global _NC_CACHE
    if _NC_CACHE is None:
        _NC_CACHE = _build_nc()
    return _NC_CACHE


def _bf(a):
    return np.asarray(a, np.float32).astype(ml_dtypes.bfloat16)


def _prepare_in_maps(inputs):
    inp = {k: np.asarray(v, np.float32) for k, v in inputs.items()}

    scale = 1.0 / np.sqrt(HD)
    common = {}
    common['w1x1'] = _bf(_lhsT(inp['conv_w'][:, :, 0]))
    common['wn0'] = _bf(np.stack([_lhsT(inp['neck_w0'][:, :, tp]) for tp in range(3)]))
    common['wnk'] = _bf(np.stack([np.stack([_lhsT(inp['neck_w'][lv][:, :, tp]) for tp in range(3)])
                                  for lv in range(3)]))
    cnb = np.zeros((128, 10), np.float32)
    cnb[:, 0:2] = _chunk_p(inp['conv_b'])
    cnb[:, 2:4] = _chunk_p(inp['neck_b0'])
    for i in range(3):
        cnb[:, 4 + 2 * i:6 + 2 * i] = _chunk_p(inp['neck_b'][i])
    common['cnb'] = cnb

    sa_w = []
    lbs = np.zeros((NLY, 128, LB_W), np.float32)
    for l in range(NLY):
        w = inp['sa_in_w'][l].copy()    # [768, 256]
        b = inp['sa_in_b'][l].copy()
        w[:COUT] *= scale
        b[:COUT] *= scale
        beta3_prev = inp['ln3_b'][l - 1] if l > 0 else np.zeros(COUT, np.float32)
        beta1 = inp['ln1_b'][l]
        beta2 = inp['ln2_b'][l]
        # qkv bias + W_qkv @ beta3(prev layer)   (fT input lacks beta3)
        b_eff = b + w @ beta3_prev
        sa_w.append(_lhsT(w))           # [2, 128, 768]
        for wi in range(3):
            lbs[l, :, LB_SA + 2 * wi:LB_SA + 2 * wi + 2] = _chunk_p(b_eff[wi * COUT:(wi + 1) * COUT])
        lbs[l, :, LB_SAO:LB_SAO + 2] = _chunk_p(inp['sa_out_b'][l] + beta3_prev)
        wq_ca = inp['ca_in_w'][l][:COUT] * scale
        lbs[l, :, LB_CAQ:LB_CAQ + 2] = _chunk_p(inp['ca_in_b'][l][:COUT] * scale + wq_ca @ beta1)
        lbs[l, :, LB_CAK:LB_CAK + 2] = _chunk_p(inp['ca_in_b'][l][COUT:2 * COUT])
        # out bias + folded V bias + beta1 (residual f1 lacks beta1)
        lbs[l, :, LB_CAO:LB_CAO + 2] = _chunk_p(
            inp['ca_out_b'][l] + inp['ca_out_w'][l] @ inp['ca_in_b'][l][2 * COUT:] + beta1)
        lbs[l, :, LB_FF1:LB_FF1 + 8] = _chunk_p(inp['ff1_b'][l] + inp['ff1_w'][l] @ beta2)
        lbs[l, :, LB_FF2:LB_FF2 + 2] = _chunk_p(inp['ff2_b'][l] + beta2)
        for wi, g in enumerate((inp['ln1_g'][l], inp['ln2_g'][l], inp['ln3_g'][l])):
            lbs[l, :, LB_G + 2 * wi:LB_G + 2 * wi + 2] = _chunk_p(g)
        lbs[l, :, LB_B3:LB_B3 + 2] = _chunk_p(inp['ln3_b'][l])
    common['lb'] = lbs
    common['sa_w'] = _bf(np.stack(sa_w))
    common['sa_ow'] = _bf(np.stack([_lhsT(inp['sa_out_w'][l]) for l in range(NLY)]))
    common['ca_qw'] = _bf(np.stack([_lhsT(inp['ca_in_w'][l][:COUT] * scale) for l in range(NLY)]))
    common['ca_kw'] = _bf(np.stack([_lhsT(inp['ca_in_w'][l][COUT:2 * COUT]) for l in range(NLY)]))
    ca_vw = []
    for l in range(NLY):
        wT = _lhsT(inp['ca_in_w'][l][2 * COUT:])          # [2, 128, 256]
        waug = np.zeros((2, 128, 512), np.float32)
        for hh2 in range(H):
            waug[:, :, 64 * hh2:64 * hh2 + 32] = wT[:, :, 32 * hh2:32 * hh2 + 32]
        ca_vw.append(waug)
    common['ca_vw'] = _bf(np.stack(ca_vw))
    common['ca_ow'] = _bf(np.stack([_lhsT(inp['ca_out_w'][l]) for l in range(NLY)]))
    common['ff1w'] = _bf(np.stack([_lhsT(inp['ff1_w'][l]) for l in range(NLY)]))
    common['ff2w'] = _bf(np.stack([_lhsT(inp['ff2_w'][l]) for l in range(NLY)]))

    metas = [_core_meta(c) for c in range(4)]
    in_maps = []
    for core in range(8):
        b, c = core // 4, core % 4
        S, pe_plus, smask = metas[c]
        xp = np.zeros((CIN, XP), np.float32)
        xp[:, 1:1 + T] = inp['x'][b]
        m = dict(common)
        m['xp'] = _bf(xp.reshape(4, 128, XP))
        m['ssel'] = _bf(S.reshape(NBLK, 128, RW))
        m['pe'] = _bf(pe_plus.reshape(2, 128, RW).transpose(1, 0, 2))
        m['smask'] = _bf(smask)
        in_maps.append(m)
    return in_maps


def kernel(**inputs):
    nc = _get_nc()
    in_maps = _prepare_in_maps(inputs)

    global LAST_EXEC_NS
    trace = bool(int(os.environ.get('KERN_TRACE', '0')))
    res = run_bass_kernel_spmd(nc, in_maps, list(range(8)), trace=trace)
    if res.exec_time_ns is not None:
        LAST_EXEC_NS = res.exec_time_ns

    out = np.zeros((B, COUT, TOT), np.float32)
    for core in range(8):
        b, c = core // 4, core % 4
        o = res.results[core]['out']          # [128, 2, RW]
        fT = o.transpose(1, 0, 2).reshape(COUT, RW)
        out[b, :, OWN * c:OWN * (c + 1)] = fT[:, HALO:HALO + OWN]
    return out


def timeline_estimate():
    """Cost-model single-core timeline estimate (ns)."""
    from concourse.timeline_sim import TimelineSim
    nc = _get_nc()
    ts = TimelineSim(nc, trace=False)
    ts.simulate()
    return ts


# revision 44
# speedup vs baseline: 1.8553x; 1.0177x over previous
"""Trainium2 Bass kernel for nn_AttnFPN (conv pyramid + 4-layer transformer
decoder with banded self-attention + dense cross-attention over a conv memory).

Sharding: 8 cores = 2 batches x 4 window-quarters of the concatenated pyramid
row space (1920 rows). Each core computes the full conv pyramid for its batch,
selects a 512-row window (480 owned rows + 16-row halo each side) via a
one-hot selection matmul, runs all 4 decoder layers on the window (halo
shrink absorbs the banded self-attention's +-4 reach per layer), and emits its
480 owned rows. The host assembles the [B, 256, 1920] output.

On-chip layout is feature-major throughout: activations live as X^T
[d on partitions (2x128 chunks), rows on free dim]. All matmul operands are
bf16 (PE: 1 cycle/row vs 4 for fp32); PSUM accumulation stays fp32. Softmax
runs without max subtraction (scores empirically bounded; exp(-1e9)
underflows to 0 for the band mask). Softmax denominators are produced as
32-row broadcasts by interleaved all-ones columns in the V stationary tiles
(ones written by memset; cross-attn V bias is folded into the output-proj
bias on the host). LayerNorm stats use all-(1/256) stationary matmuls over
feature partitions, with full-width (128-broadcast) var/sqrt/reciprocal; LN
betas are folded into downstream projection biases host-side. The K/V
projections for layer l+1 are software-pipelined into layer l's Act-bound
cross-attention window; pyramid-level selection overlaps the next level's
conv via disjoint PSUM banks.
"""
import os
import sys

for _p in ('/opt/trn_rl_repo', os.path.expanduser('~/.axon_site/_ro/trn_rl_repo')):
    if os.path.isdir(_p) and _p not in sys.path:
        sys.path.insert(0, _p)

import ml_dtypes
import numpy as np

import concourse.bass as bass
import concourse.mybir as mybir
import concourse.tile as tile
from concourse import bacc
from concourse.bass_utils import run_bass_kernel_spmd
from concourse.masks import make_identity

F32 = mybir.dt.float32
BF16 = mybir.dt.bfloat16
AF = mybir.ActivationFunctionType
OP = mybir.AluOpType

# problem constants
B, CIN, COUT, T, NLV, NLY, H, DFF, KBAND = 2, 512, 256, 2048, 4, 4, 8, 1024, 9
HD = COUT // H           # 32
RW = 512                 # per-core window rows
OWN = 480
HALO = 16
LVL_SIZES = [1024, 512, 256, 128]
LVL_STARTS = [0, 1024, 1536, 1792]
TOT = 1920
NBLK = TOT // 128        # 15 row-blocks of the concat pyramid
XP = 2056                # padded x length (col j holds x[:, j-1], col 0 = zero)
# self-attn subtiles: (q_start, q_len, k_start) window-local
SUBTILES = [(0, 120, 0), (120, 120, 116), (240, 120, 236), (360, 120, 356), (480, 32, 384)]

# packed per-layer bias/param columns (t_lb)
LB_SA, LB_SAO, LB_CAQ, LB_CAK, LB_CAO = 0, 6, 8, 10, 12
LB_FF1, LB_FF2, LB_G, LB_B3 = 14, 22, 24, 30
LB_W = 32

# ---------------------------------------------------------------------------
# device program
# ---------------------------------------------------------------------------


def _build_nc():
    nc = bacc.Bacc("TRN2", target_bir_lowering=False, debug=False, num_devices=8)

    def din(name, shape, dt=BF16):
        return nc.dram_tensor(name, list(shape), dt, kind="ExternalInput")

    t_x = din("xp", [4, 128, XP])                 # x padded, feature chunks
    t_ssel = din("ssel", [NBLK, 128, RW])         # one-hot selection
    t_pe = din("pe", [128, 2, RW])                # sqrt(C)*0 + PE slice, chunked
    t_smask = din("smask", [5, 128, 256])         # additive self masks per subtile
    # conv weights (lhsT layouts [i-chunk 128, o])
    t_w1x1 = din("w1x1", [4, 128, 256])
    t_wn0 = din("wn0", [3, 4, 128, 256])          # tap, cc, 128, 256
    t_wnk = din("wnk", [3, 3, 2, 128, 256])       # lvl-1, tap, cc, 128, 256
    t_cnb = din("cnb", [128, 10], F32)            # conv_b (2) + neck biases (4x2)
    # per-layer transformer weights
    t_sa_w = din("sa_w", [NLY, 2, 128, 768])      # qkv (q pre-scaled)
    t_sa_ow = din("sa_ow", [NLY, 2, 128, 256])
    t_ca_qw = din("ca_qw", [NLY, 2, 128, 256])    # pre-scaled
    t_ca_kw = din("ca_kw", [NLY, 2, 128, 256])
    t_ca_vw = din("ca_vw", [NLY, 2, 128, 512])    # head-interleaved, ones cols zero
    t_ca_ow = din("ca_ow", [NLY, 2, 128, 256])
    t_ff1w = din("ff1w", [NLY, 2, 128, 1024])
    t_ff2w = din("ff2w", [NLY, 8, 128, 256])
    t_lb = din("lb", [NLY, 128, LB_W], F32)       # packed biases/gammas (betas folded)
    t_out = nc.dram_tensor("out", [128, 2, RW], F32, kind="ExternalOutput")

    with tile.TileContext(nc) as tc:
        _emit(nc, tc, locals())
    nc.compile()
    return nc


def _emit(nc, tc, t):
    from contextlib import ExitStack
    ctx = ExitStack()
    with ctx:
        P = 128
        persist = ctx.enter_context(tc.tile_pool(name="persist", bufs=1))
        state = ctx.enter_context(tc.tile_pool(name="state", bufs=5))
        big = ctx.enter_context(tc.tile_pool(name="big", bufs=2))
        kvp = ctx.enter_context(tc.tile_pool(name="kvp", bufs=2))
        wroll = ctx.enter_context(tc.tile_pool(name="wroll", bufs=2))
        wb = ctx.enter_context(tc.tile_pool(name="wb", bufs=2))
        work = ctx.enter_context(tc.tile_pool(name="work", bufs=2))
        stats = ctx.enter_context(tc.tile_pool(name="stats", bufs=1))
        act = ctx.enter_context(tc.tile_pool(name="act", bufs=1))
        epool = ctx.enter_context(tc.tile_pool(name="epool", bufs=4))
        psc = ctx.enter_context(tc.tile_pool(name="psc", bufs=2, space="PSUM"))
        pav = ctx.enter_context(tc.tile_pool(name="pav", bufs=2, space="PSUM"))
        pms = ctx.enter_context(tc.tile_pool(name="pms", bufs=2, space="PSUM"))

        def stride2(ap3, cc, s, w):
            return ap3[:, cc, s:s + 2 * w].rearrange("p (n two) -> p two n", two=2)[:, 0, :]

        # ---- constants ----
        ident = persist.tile([P, P], F32)
        make_identity(nc, ident[:])
        ident_b = persist.tile([P, P], BF16)
        nc.vector.tensor_copy(ident_b[:], ident[:])
        invn = persist.tile([P, P], BF16)
        nc.gpsimd.memset(invn[:], 1.0 / COUT)
        eps_t = persist.tile([P, 1], F32)
        nc.gpsimd.memset(eps_t[:], 1e-5)

        pe_sb = persist.tile([P, 2, RW], BF16)
        smask_sb = persist.tile([P, 5, 256], BF16)

        memT = persist.tile([P, 2, T], BF16)

        import os as _os
        _nlayers = int(_os.environ.get('KERN_NLAYERS', str(NLY)))
        _stage = _os.environ.get('KERN_STAGE', 'all')

        def kv_steps(l, lb_l):
            """Yield KV-projection work for layer l in small chunks so it can
            be interleaved under Act-bound phases. First yields (KT, Vp)."""
            KT = kvp.tile([P, 2, T], BF16, tag="KT", name=f"KT{l}")
            Vp = kvp.tile([P, 16, 512], BF16, tag="Vp", name=f"Vp{l}")
            wk = wroll.tile([P, 2, 256], BF16, tag="wcak")
            nc.sync.dma_start(out=wk[:], in_=t['t_ca_kw'][l].rearrange("c p f -> p c f"))
            wv = wroll.tile([P, 2, 512], BF16, tag="wv")
            nc.sync.dma_start(out=wv[:], in_=t['t_ca_vw'][l].rearrange("c p f -> p c f"))
            Vp_v = Vp[:].rearrange("p k (h two j) -> p k h two j", h=8, two=2)
            nc.gpsimd.memset(Vp_v[:, :, :, 1, :], 1.0)
            yield (KT, Vp)
            for oc in range(2):
                for q in range(4):
                    ps = pms.tile([P, 512], F32, tag="m")
                    for ic in range(2):
                        nc.tensor.matmul(ps[:], wk[:, ic, 128 * oc:128 * (oc + 1)],
                                         memT[:, ic, 512 * q:512 * (q + 1)],
                                         start=(ic == 0), stop=(ic == 1))
                    nc.vector.tensor_scalar_add(KT[:, oc, 512 * q:512 * (q + 1)],
                                                ps[:], lb_l[:, LB_CAK + oc:LB_CAK + oc + 1])
                    yield None
            for kc in range(16):
                ps = pms.tile([P, 512], F32, tag="m")
                for ic in range(2):
                    nc.tensor.matmul(ps[:], memT[:, ic, 128 * kc:128 * (kc + 1)],
                                     wv[:, ic, :], start=(ic == 0), stop=(ic == 1))
                ps_v = ps[:].rearrange("p (h two j) -> p h two j", h=8, two=2)
                nc.vector.tensor_copy(Vp_v[:, kc, :, 0, :], ps_v[:, :, 0, :])
                if kc % 2 == 1:
                    yield None

        def drain(gen):
            if gen is not None:
                for _ in gen:
                    pass


        # ================= pyramid =================
        with tc.tile_pool(name="pyr", bufs=1) as pyr:
            xT = pyr.tile([P, 4, XP], BF16)
            w1x1 = pyr.tile([P, 4, 256], BF16)
            nc.sync.dma_start(out=xT[:, 0, :], in_=t['t_x'][0])
            nc.sync.dma_start(out=w1x1[:], in_=t['t_w1x1'].ap().rearrange("c p f -> p c f"))
            for cc in range(1, 4):
                nc.sync.dma_start(out=xT[:, cc, :], in_=t['t_x'][cc])
            cnb = pyr.tile([P, 10], F32)
            nc.sync.dma_start(out=cnb[:], in_=t['t_cnb'].ap())
            wn0 = pyr.tile([P, 12, 256], BF16)
            nc.sync.dma_start(out=wn0[:], in_=t['t_wn0'].ap().rearrange("t c p f -> p (t c) f"))
            wnk = pyr.tile([P, 18, 256], BF16)
            nc.sync.dma_start(out=wnk[:], in_=t['t_wnk'].ap().rearrange("l t c p f -> p (l t c) f"))
            ssel = pyr.tile([P, NBLK, RW], BF16)
            nc.sync.dma_start(out=ssel[:], in_=t['t_ssel'].ap().rearrange("b p f -> p b f"))
            lbs_t = []
            for l in range(_nlayers):
                lb_l = wb.tile([P, LB_W], F32, tag="lb", name=f"lb{l}")
                nc.sync.dma_start(out=lb_l[:], in_=t['t_lb'][l])
                lbs_t.append(lb_l)
            kv_gen = kv_steps(0, lbs_t[0]) if _nlayers else None
            kv_tiles = next(kv_gen) if kv_gen else None
            nc.sync.dma_start(out=pe_sb[:], in_=t['t_pe'].ap())
            nc.sync.dma_start(out=smask_sb[:], in_=t['t_smask'].ap().rearrange("s p f -> p s f"))

            # mem = relu(1x1 conv), fc processed in pairs (2 psum slots)
            for oc in range(2):
                for fp in range(2):
                    pss = [pms.tile([P, 512], F32, tag="m", name=f"mempp{oc}_{fp}_{i2}") for i2 in range(2)]
                    for cc in range(4):
                        wsl = w1x1[:, cc, 128 * oc:128 * (oc + 1)]
                        for i, fc in enumerate((2 * fp, 2 * fp + 1)):
                            nc.tensor.matmul(pss[i][:], wsl,
                                             xT[:, cc, 1 + 512 * fc:1 + 512 * (fc + 1)],
                                             start=(cc == 0), stop=(cc == 3))
                    for i, fc in enumerate((2 * fp, 2 * fp + 1)):
                        nc.vector.tensor_scalar(out=memT[:, oc, 512 * fc:512 * (fc + 1)],
                                                in0=pss[i][:], scalar1=cnb[:, oc:oc + 1],
                                                scalar2=0.0, op0=OP.add, op1=OP.max)

            # neck pyramid (feature-major, 1-col zero pad left); each level's
            # transpose+selection blocks are emitted right after the level's
            # conv so they overlap the next level's conv (disjoint PSUM tags:
            # conv='m', transposes='sc', selection accumulators='av')
            lvl_len = [1024, 512, 256, 128]
            sel_ps = [pav.tile([P, 512], F32, tag="av", name=f"selps{dc}") for dc in range(2)]
            f0 = state.tile([P, 2, RW], BF16, tag="fT")
            blk0 = [0, 8, 12, 14]
            src_buf = None
            for lv in range(4):
                L = lvl_len[lv]
                lb_t = pyr.tile([P, 2, L + 8], BF16, tag=f"lb{lv}", name=f"lb{lv}")
                nc.gpsimd.memset(lb_t[:], 0.0)
                n_cc = 4 if lv == 0 else 2
                nfc = (L + 511) // 512
                for oc in range(2):
                    pss = [pms.tile([P, 512], F32, tag="m", name=f"cvp{lv}_{oc}_{i2}") for i2 in range(nfc)]
                    k = 0
                    for cc in range(n_cc):
                        for tap in range(3):
                            if lv == 0:
                                wsl = wn0[:, 4 * tap + cc, 128 * oc:128 * (oc + 1)]
                            else:
                                wsl = wnk[:, 6 * (lv - 1) + 2 * tap + cc, 128 * oc:128 * (oc + 1)]
                            for fc in range(nfc):
                                w = min(512, L - 512 * fc)
                                rhs = (stride2(xT, cc, 1024 * fc + tap, w) if lv == 0
                                       else stride2(src_buf, cc, 1024 * fc + tap, w))
                                nc.tensor.matmul(pss[fc][:, :w], wsl, rhs,
                                                 start=(k == 0), stop=(k == 3 * n_cc - 1))
                            k += 1
                    for fc in range(nfc):
                        w = min(512, L - 512 * fc)
                        nc.vector.tensor_scalar(out=lb_t[:, oc, 1 + 512 * fc:1 + 512 * fc + w],
                                                in0=pss[fc][:, :w], scalar1=cnb[:, 2 + 2 * lv + oc:3 + 2 * lv + oc],
                                                scalar2=0.0, op0=OP.add, op1=OP.max)
                src_buf = lb_t
                for j in range(L // 128):
                    b = blk0[lv] + j
                    for dc in range(2):
                        tr_ps = psc.tile([P, P], BF16, tag="sc")
                        nc.tensor.transpose(tr_ps[:],
                                            lb_t[:, dc, 1 + 128 * j:1 + 128 * (j + 1)],
                                            ident_b[:])
                        fr = work.tile([P, P], BF16, tag="frow")
                        nc.vector.tensor_copy(fr[:], tr_ps[:])
                        nc.tensor.matmul(sel_ps[dc][:], fr[:], ssel[:, b, :],
                                         start=(b == 0), stop=(b == NBLK - 1))
                    if kv_gen is not None:
                        next(kv_gen, None)
            drain(kv_gen)
            kv_gen = None
            for dc in range(2):
                nc.vector.scalar_tensor_tensor(out=f0[:, dc, :], in0=sel_ps[dc][:],
                                               scalar=float(np.sqrt(COUT)),
                                               in1=pe_sb[:, dc, :],
                                               op0=OP.mult, op1=OP.add)

        # ================= decoder layers =================
        fT = f0

        def _unused_kv_steps(l, lb_l):
            """Yield KV-projection work for layer l in small chunks so it can
            be interleaved under the Act-bound cross-attn of layer l-1.
            Yields after every few PE ops; first yields (KT, Vp) tiles."""
            KT = kvp.tile([P, 2, T], BF16, tag="KT", name=f"KT{l}")
            Vp = kvp.tile([P, 16, 512], BF16, tag="Vp", name=f"Vp{l}")
            wk = wroll.tile([P, 2, 256], BF16, tag="wcak")
            nc.sync.dma_start(out=wk[:], in_=t['t_ca_kw'][l].rearrange("c p f -> p c f"))
            wv = wroll.tile([P, 2, 512], BF16, tag="wv")
            nc.sync.dma_start(out=wv[:], in_=t['t_ca_vw'][l].rearrange("c p f -> p c f"))
            Vp_v = Vp[:].rearrange("p k (h two j) -> p k h two j", h=8, two=2)
            nc.gpsimd.memset(Vp_v[:, :, :, 1, :], 1.0)
            yield (KT, Vp)
            for oc in range(2):
                for q in range(4):
                    ps = pms.tile([P, 512], F32, tag="m")
                    for ic in range(2):
                        nc.tensor.matmul(ps[:], wk[:, ic, 128 * oc:128 * (oc + 1)],
                                         memT[:, ic, 512 * q:512 * (q + 1)],
                                         start=(ic == 0), stop=(ic == 1))
                    nc.vector.tensor_scalar_add(KT[:, oc, 512 * q:512 * (q + 1)],
                                                ps[:], lb_l[:, LB_CAK + oc:LB_CAK + oc + 1])
                    yield None
            for kc in range(16):
                ps = pms.tile([P, 512], F32, tag="m")
                for ic in range(2):
                    nc.tensor.matmul(ps[:], memT[:, ic, 128 * kc:128 * (kc + 1)],
                                     wv[:, ic, :], start=(ic == 0), stop=(ic == 1))
                ps_v = ps[:].rearrange("p (h two j) -> p h two j", h=8, two=2)
                nc.vector.tensor_copy(Vp_v[:, kc, :, 0, :], ps_v[:, :, 0, :])
                if kc % 2 == 1:
                    yield None

        def drain(gen):
            if gen is not None:
                for _ in gen:
                    pass

        for l in range(_nlayers):
            lb = lbs_t[l]
            KT, Vp = kv_tiles
            if l + 1 < _nlayers:
                kv_gen = kv_steps(l + 1, lbs_t[l + 1])
                kv_tiles = next(kv_gen)
            else:
                kv_gen = None

            if _stage == 'kv':
                drain(kv_gen)
                continue
            # ---- self attention ----
            wsa = wroll.tile([P, 2, 768], BF16, tag="wsa")
            nc.sync.dma_start(out=wsa[:], in_=t['t_sa_w'][l].rearrange("c p f -> p c f"))
            QTs = act.tile([P, 2, RW], BF16, tag="QTs")
            KTs = act.tile([P, 2, RW], BF16, tag="KTs")
            VTs = act.tile([P, 2, RW], BF16, tag="VTs")
            for wi, dst in ((0, QTs), (1, KTs), (2, VTs)):
                for oc in range(2):
                    ps = pms.tile([P, 512], F32, tag="m")
                    for ic in range(2):
                        nc.tensor.matmul(ps[:], wsa[:, ic, 256 * wi + 128 * oc:256 * wi + 128 * (oc + 1)],
                                         fT[:, ic, :],
                                         start=(ic == 0), stop=(ic == 1))
                    nc.vector.tensor_scalar_add(dst[:, oc, :], ps[:],
                                                lb[:, LB_SA + 2 * wi + oc:LB_SA + 2 * wi + oc + 1])
            OsT = act.tile([P, 2, RW], BF16, tag="OT")
            wsao = wroll.tile([P, 2, 256], BF16, tag="wsao")
            nc.sync.dma_start(out=wsao[:], in_=t['t_sa_ow'][l].rearrange("c p f -> p c f"))
            R1 = state.tile([P, 2, RW], BF16, tag="fT")
            for sti, (qs, ql, ks) in enumerate(SUBTILES):
                # vst: transposed V block [keys, 8*32 vdims] + 32 ones cols
                vst = work.tile([P, 288], BF16, tag="vst")
                for hc in range(2):
                    vst_ps = pms.tile([P, 128], BF16, tag="m")
                    nc.tensor.transpose(vst_ps[:],
                                        VTs[:, hc, ks:ks + 128], ident_b[:])
                    nc.vector.tensor_copy(vst[:, 128 * hc:128 * (hc + 1)], vst_ps[:])
                nc.gpsimd.memset(vst[:, 256:288], 1.0)
                # all 8 heads' scores (+mask) in one PSUM tile, one exp
                sps = psc.tile([P, 1024], F32, tag="sc")
                for hh in range(8):
                    nc.tensor.matmul(sps[:, 128 * hh:128 * hh + ql],
                                     KTs[32 * (hh % 4):32 * (hh % 4) + 32, hh // 4, ks:ks + 128],
                                     QTs[32 * (hh % 4):32 * (hh % 4) + 32, hh // 4, qs:qs + ql],
                                     start=True, stop=False,
                                     tile_position=(32 * (hh % 4), 0))
                    nc.tensor.matmul(sps[:, 128 * hh:128 * hh + ql], ident_b[:],
                                     smask_sb[:, sti, 0:ql],
                                     start=False, stop=True)
                es = epool.tile([P, 1024], BF16, tag="E")
                nc.scalar.activation(out=es[:].rearrange("p (h q) -> p h q", h=8)[:, :, 0:ql],
                                     in_=sps[:].rearrange("p (h q) -> p h q", h=8)[:, :, 0:ql],
                                     func=AF.Exp)
                # avp: rows 0:64 = AV (2 heads), 64:128 = denominators; cols 128p
                avp = pav.tile([P, 512], F32, tag="av")
                for p in range(4):
                    h0, h1 = 2 * p, 2 * p + 1
                    nc.tensor.matmul(avp[0:32, 128 * p:128 * p + ql], vst[:, 64 * p:64 * p + 32],
                                     es[:, 128 * h0:128 * h0 + ql], start=True, stop=True,
                                     tile_position=(0, 0))
                    nc.tensor.matmul(avp[32:64, 128 * p:128 * p + ql], vst[:, 64 * p + 32:64 * p + 64],
                                     es[:, 128 * h1:128 * h1 + ql], start=True, stop=True,
                                     tile_position=(0, 32))
                    nc.tensor.matmul(avp[64:96, 128 * p:128 * p + ql], vst[:, 256:288],
                                     es[:, 128 * h0:128 * h0 + ql], start=True, stop=True,
                                     tile_position=(0, 64))
                    nc.tensor.matmul(avp[96:128, 128 * p:128 * p + ql], vst[:, 256:288],
                                     es[:, 128 * h1:128 * h1 + ql], start=True, stop=True,
                                     tile_position=(0, 96))
                zr = work.tile([P, 512], F32, tag="zr")
                nc.vector.reciprocal(zr[64:128, :], avp[64:128, :])
                for p in range(4):
                    nc.vector.tensor_mul(OsT[64 * (p % 2):64 * (p % 2) + 64, p // 2, qs:qs + ql],
                                         avp[0:64, 128 * p:128 * p + ql],
                                         zr[64:128, 128 * p:128 * p + ql])
            for oc in range(2):
                ps = pms.tile([P, 512], F32, tag="m")
                for ic in range(2):
                    nc.tensor.matmul(ps[:], wsao[:, ic, 128 * oc:128 * (oc + 1)], OsT[:, ic, :],
                                     start=(ic == 0), stop=(ic == 1))
                nc.vector.scalar_tensor_tensor(out=R1[:, oc, :], in0=ps[:],
                                               scalar=lb[:, LB_SAO + oc:LB_SAO + oc + 1], in1=fT[:, oc, :],
                                               op0=OP.add, op1=OP.add)
            if kv_gen is not None:
                for _ in range(3):
                    next(kv_gen, None)
            f1 = state.tile([P, 2, RW], BF16, tag="fT")
            _layernorm(nc, pms, work, stats, act, R1, f1, lb, 0, invn, eps_t)

            if _stage == 'self':
                drain(kv_gen)
                fT = f1
                continue
            # ---- cross attention: two q-halves, 4-head quads per exp
            # slice; the A-half post-chain (R2/LN2/FFN/LN3 cols 0:256) is
            # pumped under the B-half's Act-bound window ----
            wcaq = wroll.tile([P, 2, 256], BF16, tag="wcaq")
            nc.sync.dma_start(out=wcaq[:], in_=t['t_ca_qw'][l].rearrange("c p f -> p c f"))
            wcao = wroll.tile([P, 2, 256], BF16, tag="wcao")
            nc.sync.dma_start(out=wcao[:], in_=t['t_ca_ow'][l].rearrange("c p f -> p c f"))
            wf1 = wroll.tile([P, 2, 1024], BF16, tag="wf1")
            nc.sync.dma_start(out=wf1[:], in_=t['t_ff1w'][l].rearrange("c p f -> p c f"))
            wf2 = wroll.tile([P, 8, 256], BF16, tag="wf2")
            nc.sync.dma_start(out=wf2[:], in_=t['t_ff2w'][l].rearrange("c p f -> p c f"))
            QTc = act.tile([P, 2, RW], BF16, tag="QTc")
            for oc in range(2):
                ps = pms.tile([P, 512], F32, tag="m")
                for ic in range(2):
                    nc.tensor.matmul(ps[:], wcaq[:, ic, 128 * oc:128 * (oc + 1)], f1[:, ic, :],
                                     start=(ic == 0), stop=(ic == 1))
                nc.vector.tensor_scalar_add(QTc[:, oc, :], ps[:], lb[:, LB_CAQ + oc:LB_CAQ + oc + 1])
            OcT = act.tile([P, 2, RW], BF16, tag="OT")
            R2 = state.tile([P, 2, RW], BF16, tag="fT")
            f2 = state.tile([P, 2, RW], BF16, tag="fT")
            R3 = state.tile([P, 2, RW], BF16, tag="fT")
            f3 = state.tile([P, 2, RW], BF16, tag="fT")

            def emit_r2(c0, c1):
                w = c1 - c0
                for oc in range(2):
                    ps = pms.tile([P, 512], F32, tag="m")
                    for ic in range(2):
                        nc.tensor.matmul(ps[:, 0:w], wcao[:, ic, 128 * oc:128 * (oc + 1)],
                                         OcT[:, ic, c0:c1], start=(ic == 0), stop=(ic == 1))
                    nc.vector.scalar_tensor_tensor(out=R2[:, oc, c0:c1], in0=ps[:, 0:w],
                                                   scalar=lb[:, LB_CAO + oc:LB_CAO + oc + 1],
                                                   in1=f1[:, oc, c0:c1], op0=OP.add, op1=OP.add)

            def ffn_steps(c0, c1):
                """FFN1 -> FFN2 -> R3 for columns [c0, c1). No Act-engine ops,
                so it can pump under cross-attn without table switches."""
                w = c1 - c0
                ps_oc = [pms.tile([P, 512], F32, tag="m", name=f"ffp{l}_{i2}") for i2 in range(2)]
                for hf in range(2):
                    Hb = big.tile([P, 4, 512], BF16, tag="Hb")
                    for jj in range(4):
                        j = 4 * hf + jj
                        ps = psc.tile([P, 1024], F32, tag="sc")
                        for ic in range(2):
                            nc.tensor.matmul(ps[:, 0:w], wf1[:, ic, 128 * j:128 * (j + 1)],
                                             f2[:, ic, c0:c1], start=(ic == 0), stop=(ic == 1))
                        nc.vector.tensor_scalar(out=Hb[:, jj, 0:w], in0=ps[:, 0:w],
                                                scalar1=lb[:, LB_FF1 + j:LB_FF1 + j + 1], scalar2=0.0,
                                                op0=OP.add, op1=OP.max)
                        yield None
                    for jj in range(4):
                        j = 4 * hf + jj
                        for oc in range(2):
                            nc.tensor.matmul(ps_oc[oc][:, 0:w], wf2[:, j, 128 * oc:128 * (oc + 1)],
                                             Hb[:, jj, 0:w], start=(j == 0), stop=(j == 7))
                        yield None
                for oc in range(2):
                    nc.vector.scalar_tensor_tensor(out=R3[:, oc, c0:c1], in0=ps_oc[oc][:, 0:w],
                                                   scalar=lb[:, LB_FF2 + oc:LB_FF2 + oc + 1],
                                                   in1=f2[:, oc, c0:c1], op0=OP.add, op1=OP.add)
                    yield None

            for p in range(4):
                h0, h1 = 2 * p, 2 * p + 1
                avp = pav.tile([P, 512], F32, tag="av")
                for kc in range(16):
                    scp = psc.tile([P, 1024], F32, tag="sc")
                    for hi, hh in enumerate((h0, h1)):
                        nc.tensor.matmul(scp[:, 512 * hi:512 * (hi + 1)],
                                         KT[32 * (hh % 4):32 * (hh % 4) + 32, hh // 4, 128 * kc:128 * (kc + 1)],
                                         QTc[32 * (hh % 4):32 * (hh % 4) + 32, hh // 4, :],
                                         start=True, stop=True, tile_position=(32 * (hh % 4), 0))
                    ec = epool.tile([P, 1024], BF16, tag="E")
                    nc.scalar.activation(out=ec[:], in_=scp[:], func=AF.Exp)
                    st, sp = (kc == 0), (kc == 15)
                    nc.tensor.matmul(avp[0:64, :], Vp[:, kc, 64 * h0:64 * h0 + 64],
                                     ec[:, 0:512], start=st, stop=sp, tile_position=(0, 0))
                    nc.tensor.matmul(avp[64:128, :], Vp[:, kc, 64 * h1:64 * h1 + 64],
                                     ec[:, 512:1024], start=st, stop=sp, tile_position=(0, 64))
                    if kv_gen is not None and kc % 4 == 3:
                        next(kv_gen, None)
                zr = work.tile([P, 512], F32, tag="zr")
                nc.vector.reciprocal(zr[:, :], avp[:, :])
                nc.vector.tensor_mul(OcT[64 * (p % 2):64 * (p % 2) + 32, p // 2, :],
                                     avp[0:32, :], zr[32:64, :])
                nc.vector.tensor_mul(OcT[64 * (p % 2) + 32:64 * (p % 2) + 64, p // 2, :],
                                     avp[64:96, :], zr[96:128, :])
            drain(kv_gen)
            emit_r2(0, RW)
            _layernorm(nc, pms, work, stats, act, R2, f2, lb, 1, invn, eps_t)
            for _ in ffn_steps(0, RW):
                pass
            _layernorm(nc, pms, work, stats, act, R3, f3, lb, 2, invn, eps_t)
            fT = f3

        # final: add back LN3 beta (unfolded) and emit fp32
        outf = act.tile([P, 2, RW], F32, tag="outf")
        for oc in range(2):
            nc.vector.tensor_scalar_add(outf[:, oc, :], fT[:, oc, :],
                                        lb[:, LB_B3 + oc:LB_B3 + oc + 1])
            nc.sync.dma_start(out=t['t_out'].ap()[:, oc, :], in_=outf[:, oc, :])


def _layernorm(nc, pms, work, stats, act, R, out, lb, which, invn, eps_t, c0=0, c1=RW):
    """Feature-major LN over d=256 (2 partition chunks), rows c0:c1 on the
    free dim. Stats via all-(1/256) stationary matmuls producing 128-row
    broadcasts; gamma per-partition; beta folded into downstream biases."""
    P = 128
    w = c1 - c0
    sq = act.tile([P, 2, RW], BF16, tag="sq")
    for oc in range(2):
        nc.vector.tensor_mul(sq[:, oc, c0:c1], R[:, oc, c0:c1], R[:, oc, c0:c1])
    mB = pms.tile([P, 512], F32, tag="m")
    for ic in range(2):
        nc.tensor.matmul(mB[:, 0:w], invn[:], R[:, ic, c0:c1], start=(ic == 0), stop=(ic == 1))
    msB = pms.tile([P, 512], F32, tag="m")
    for ic in range(2):
        nc.tensor.matmul(msB[:, 0:w], invn[:], sq[:, ic, c0:c1], start=(ic == 0), stop=(ic == 1))
    sqm = stats.tile([P, RW], F32, tag="s1")
    nc.scalar.activation(out=sqm[:, 0:w], in_=mB[:, 0:w], func=AF.Square)
    varB = stats.tile([P, RW], F32, tag="s2")
    nc.vector.tensor_sub(varB[:, 0:w], msB[:, 0:w], sqm[:, 0:w])
    sdB = stats.tile([P, RW], F32, tag="s1")
    nc.scalar.activation(out=sdB[:, 0:w], in_=varB[:, 0:w], func=AF.Sqrt, bias=eps_t[:])
    zrB = stats.tile([P, RW], F32, tag="s2")
    nc.vector.reciprocal(zrB[:, 0:w], sdB[:, 0:w])
    for oc in range(2):
        # out = ((R - mB) * gamma) * zrB   (gamma per-partition; beta folded)
        c = work.tile([P, RW], F32, tag="tmp")
        nc.vector.tensor_sub(c[:, 0:w], R[:, oc, c0:c1], mB[:, 0:w])
        nc.vector.scalar_tensor_tensor(out=out[:, oc, c0:c1], in0=c[:, 0:w],
                                       scalar=lb[:, LB_G + 2 * which + oc:LB_G + 2 * which + oc + 1],
                                       in1=zrB[:, 0:w], op0=OP.mult, op1=OP.mult)


# ---------------------------------------------------------------------------
# host side
# ---------------------------------------------------------------------------

def _sinusoidal_pe(t, d):
    pos = np.arange(t, dtype=np.float32)[:, None]
    div = np.exp(np.arange(0, d, 2, dtype=np.float32) * (-np.log(10000.0) / d))
    ang = pos * div
    pe = np.zeros((t, d), np.float32)
    pe[:, 0::2] = np.sin(ang)
    pe[:, 1::2] = np.cos(ang)
    return pe


def _concat_row_to_level(r):
    for li in range(NLV):
        if r < LVL_STARTS[li] + LVL_SIZES[li]:
            return li, r - LVL_STARTS[li]
    raise ValueError(r)


def _core_meta(c):
    w0 = OWN * c - HALO
    S = np.zeros((TOT, RW), np.float32)
    valid = np.zeros(RW, bool)
    lvl_of = np.full(RW, -1)
    pos_of = np.full(RW, -1)
    for j in range(RW):
        r = w0 + j
        if 0 <= r < TOT:
            S[r, j] = 1.0
            valid[j] = True
            lvl_of[j], pos_of[j] = _concat_row_to_level(r)
    pes = [_sinusoidal_pe(sz, COUT) for sz in LVL_SIZES]
    pe_plus = np.zeros((COUT, RW), np.float32)
    for j in range(RW):
        if valid[j]:
            pe_plus[:, j] = pes[lvl_of[j]][pos_of[j]]
    smask = np.full((5, 128, 256), -1e9, np.float32)
    for sti, (qs, ql, ks) in enumerate(SUBTILES):
        m = np.full((128, ql), -1e9, np.float32)
        for jq in range(ql):
            q = qs + jq
            for jk in range(128):
                k = ks + jk
                if k >= RW:
                    continue
                if valid[q] and valid[k]:
                    if lvl_of[q] == lvl_of[k] and abs(pos_of[q] - pos_of[k]) <= KBAND // 2:
                        m[jk, jq] = 0.0
                elif (not valid[q]) and k == q:
                    m[jk, jq] = 0.0
        smask[sti, :, 0:ql] = m
        smask[sti, :, 128:128 + ql] = m
    return S, pe_plus, smask


def _chunk_p(v):
    """[n*128] -> [128, n] partition-major."""
    v = np.asarray(v, np.float32)
    n = v.shape[0] // 128
    return v.reshape(n, 128).T.copy()


def _lhsT(w):
    """[O, I] weight -> [n_ic, 128, O] lhsT chunks (W^T chunked over I)."""
    wT = np.ascontiguousarray(np.asarray(w, np.float32).T)  # [I, O]
    I = wT.shape[0]
    return wT.reshape(I // 128, 128, wT.shape[1])


_NC_CACHE = None
LAST_EXEC_NS = None


def _get_nc():
    global _NC_CACHE
    if _NC_CACHE is None:
        _NC_CACHE = _build_nc()
    return _NC_CACHE


def _bf(a):
    return np.asarray(a, np.float32).astype(ml_dtypes.bfloat16)


def _prepare_in_maps(inputs):
    inp = {k: np.asarray(v, np.float32) for k, v in inputs.items()}

    scale = 1.0 / np.sqrt(HD)
    common = {}
    common['w1x1'] = _bf(_lhsT(inp['conv_w'][:, :, 0]))
    common['wn0'] = _bf(np.stack([_lhsT(inp['neck_w0'][:, :, tp]) for tp in range(3)]))
    common['wnk'] = _bf(np.stack([np.stack([_lhsT(inp['neck_w'][lv][:, :, tp]) for tp in range(3)])
                                  for lv in range(3)]))
    cnb = np.zeros((128, 10), np.float32)
    cnb[:, 0:2] = _chunk_p(inp['conv_b'])
    cnb[:, 2:4] = _chunk_p(inp['neck_b0'])
    for i in range(3):
        cnb[:, 4 + 2 * i:6 + 2 * i] = _chunk_p(inp['neck_b'][i])
    common['cnb'] = cnb

    sa_w = []
    lbs = np.zeros((NLY, 128, LB_W), np.float32)
    for l in range(NLY):
        w = inp['sa_in_w'][l].copy()    # [768, 256]
        b = inp['sa_in_b'][l].copy()
        w[:COUT] *= scale
        b[:COUT] *= scale
        beta3_prev = inp['ln3_b'][l - 1] if l > 0 else np.zeros(COUT, np.float32)
        beta1 = inp['ln1_b'][l]
        beta2 = inp['ln2_b'][l]
        # qkv bias + W_qkv @ beta3(prev layer)   (fT input lacks beta3)
        b_eff = b + w @ beta3_prev
        sa_w.append(_lhsT(w))           # [2, 128, 768]
        for wi in range(3):
            lbs[l, :, LB_SA + 2 * wi:LB_SA + 2 * wi + 2] = _chunk_p(b_eff[wi * COUT:(wi + 1) * COUT])
        lbs[l, :, LB_SAO:LB_SAO + 2] = _chunk_p(inp['sa_out_b'][l] + beta3_prev)
        wq_ca = inp['ca_in_w'][l][:COUT] * scale
        lbs[l, :, LB_CAQ:LB_CAQ + 2] = _chunk_p(inp['ca_in_b'][l][:COUT] * scale + wq_ca @ beta1)
        lbs[l, :, LB_CAK:LB_CAK + 2] = _chunk_p(inp['ca_in_b'][l][COUT:2 * COUT])
        # out bias + folded V bias + beta1 (residual f1 lacks beta1)
        lbs[l, :, LB_CAO:LB_CAO + 2] = _chunk_p(
            inp['ca_out_b'][l] + inp['ca_out_w'][l] @ inp['ca_in_b'][l][2 * COUT:] + beta1)
        lbs[l, :, LB_FF1:LB_FF1 + 8] = _chunk_p(inp['ff1_b'][l] + inp['ff1_w'][l] @ beta2)
        lbs[l, :, LB_FF2:LB_FF2 + 2] = _chunk_p(inp['ff2_b'][l] + beta2)
        for wi, g in enumerate((inp['ln1_g'][l], inp['ln2_g'][l], inp['ln3_g'][l])):
            lbs[l, :, LB_G + 2 * wi:LB_G + 2 * wi + 2] = _chunk_p(g)
        lbs[l, :, LB_B3:LB_B3 + 2] = _chunk_p(inp['ln3_b'][l])
    common['lb'] = lbs
    common['sa_w'] = _bf(np.stack(sa_w))
    common['sa_ow'] = _bf(np.stack([_lhsT(inp['sa_out_w'][l]) for l in range(NLY)]))
    common['ca_qw'] = _bf(np.stack([_lhsT(inp['ca_in_w'][l][:COUT] * scale) for l in range(NLY)]))
    common['ca_kw'] = _bf(np.stack([_lhsT(inp['ca_in_w'][l][COUT:2 * COUT]) for l in range(NLY)]))
    ca_vw = []
    for l in range(NLY):
        wT = _lhsT(inp['ca_in_w'][l][2 * COUT:])          # [2, 128, 256]
        waug = np.zeros((2, 128, 512), np.float32)
        for hh2 in range(H):
            waug[:, :, 64 * hh2:64 * hh2 + 32] = wT[:, :, 32 * hh2:32 * hh2 + 32]
        ca_vw.append(waug)
    common['ca_vw'] = _bf(np.stack(ca_vw))
    common['ca_ow'] = _bf(np.stack([_lhsT(inp['ca_out_w'][l]) for l in range(NLY)]))
    common['ff1w'] = _bf(np.stack([_lhsT(inp['ff1_w'][l]) for l in range(NLY)]))
    common['ff2w'] = _bf(np.stack([_lhsT(inp['ff2_w'][l]) for l in range(NLY)]))

    metas = [_core_meta(c) for c in range(4)]
    in_maps = []
    for core in range(8):
        b, c = core // 4, core % 4
        S, pe_plus, smask = metas[c]
        xp = np.zeros((CIN, XP), np.float32)
        xp[:, 1:1 + T] = inp['x'][b]
        m = dict(common)
        m['xp'] = _bf(xp.reshape(4, 128, XP))
        m['ssel'] = _bf(S.reshape(NBLK, 128, RW))
        m['pe'] = _bf(pe_plus.reshape(2, 128, RW).transpose(1, 0, 2))
        m['smask'] = _bf(smask)
        in_maps.append(m)
    return in_maps


def kernel(**inputs):
    nc = _get_nc()
    in_maps = _prepare_in_maps(inputs)

    global LAST_EXEC_NS
    trace = bool(int(os.environ.get('KERN_TRACE', '0')))
    res = run_bass_kernel_spmd(nc, in_maps, list(range(8)), trace=trace)
    if res.exec_time_ns is not None:
        LAST_EXEC_NS = res.exec_time_ns

    out = np.zeros((B, COUT, TOT), np.float32)
    for core in range(8):
        b, c = core // 4, core % 4
        o = res.results[core]['out']          # [128, 2, RW]
        fT = o.transpose(1, 0, 2).reshape(COUT, RW)
        out[b, :, OWN * c:OWN * (c + 1)] = fT[:, HALO:HALO + OWN]
    return out


def timeline_estimate():
    """Cost-model single-core timeline estimate (ns)."""
    from concourse.timeline_sim import TimelineSim
    nc = _get_nc()
    ts = TimelineSim(nc, trace=False)
    ts.simulate()
    return ts
